# revision 39
# baseline (speedup 1.0000x reference)
"""Trainium2 Bass kernel for nn_BuildCostVolume (stereo cost volume + Mamba scan).

Sharding: disparity axis (24) split as 3 per core across 8 cores; core k
handles disparities d = 3k+j (j in 0..2, compile-time; host pre-shifts
featuresR by 3k so the SPMD program is identical across cores).

Per-core pipeline (software-pipelined across the 6 (j, h-half) chunks as
s4(k) | tail(k-1) | pairs01(k) | s4(k+1) | pairs23(k) | tail(k) | ...):
  - Features loaded once as bf16; u/dt/B/C/D projections on PE from L and
    shifted-R views with even/odd split weights (channel interleave trick).
  - dt = softplus via Exp + Ln(x+1) on ACT; u evicted via ACT Copy so the
    dt*u multiply runs at the DVE 2x (16-bit) rate.
  - Decay a = exp(A*dt) via ACT per-partition scale in an (s-pair x e)
    128-partition layout; B broadcast via SBUF-to-SBUF DMA (Pool queue);
    b = dt*u*B on DVE, with pairs 1,3 offloaded to GPSIMD to balance.
  - Mamba recurrence h = a*h + b via DVE tensor_tensor_scan over flattened
    (row, w) with a[w=0]=0 so each image row restarts the scan.
  - y/cost contraction on PE (block-diag W_out fold, C multiply at PSUM
    eviction, partition-sum + D-term matmul); cstg evicted on ACT with
    fused avg-pool accumulation.
  - Channel-attn max pool from the spatial S layout: masked add + per-g
    max on 128 partitions, GPSIMD partition_all_reduce, tiny transposing
    DMAs; MLP in bf16; spatial attention as in the reference.
  - Output written [j*64+h, g, w] bf16 and transposed/cast on host.
"""
import os
import numpy as np

C, H, W, DV = 32, 64, 128, 24
_NCH_ENV = int(os.environ.get("KERNEL_NCH", "6"))
_SKIP_EPI = bool(int(os.environ.get("KERNEL_SKIP_EPI", "0")))
_SKIP_PAIRS = bool(int(os.environ.get("KERNEL_SKIP_PAIRS", "0")))
_ITERS = int(os.environ.get("KERNEL_ITERS", "1"))
E, S, G = 64, 8, 8
NCORES, JD = 8, 3          # cores, disparities per core
PAD = 8                    # leading zero columns in feature tensors
HH = 32                    # h rows per chunk
NCH = 6                    # chunks = (j, h-half)
CCOLS = HH * W             # 4096 columns per chunk
HW = H * W                 # 8192
FROW = PAD + HW            # 8200 cols per feature image

_compiled = {}


def _f32(x):
    return np.ascontiguousarray(np.asarray(x, np.float32))


def _build_program():
    import concourse.bacc as bacc
    import concourse.mybir as mybir
    from concourse.tile import TileContext

    F32 = mybir.dt.float32
    BF16 = mybir.dt.bfloat16
    AF = mybir.ActivationFunctionType
    AX = mybir.AxisListType
    OP = mybir.AluOpType

    nc = bacc.Bacc("TRN2", target_bir_lowering=False, debug=False,
                   num_devices=NCORES)

    feat_d = nc.dram_tensor("feat", [C, 2 * FROW], BF16, kind="ExternalInput").ap()
    wse_d = nc.dram_tensor("wse", [2 * C, 576], BF16, kind="ExternalInput").ap()
    wbf_d = nc.dram_tensor("wbf", [128, 48], BF16, kind="ExternalInput").ap()
    avec_d = nc.dram_tensor("avec", [128, 8], F32, kind="ExternalInput").ap()
    umask_d = nc.dram_tensor("umask", [32, JD * W], BF16, kind="ExternalInput").ap()
    mnegs_d = nc.dram_tensor("mnegs", [128, 2 * G * W], BF16, kind="ExternalInput").ap()
    invc_d = nc.dram_tensor("invc", [G, JD], F32, kind="ExternalInput").ap()
    wsp_d = nc.dram_tensor("wsp", [128, 4], F32, kind="ExternalInput").ap()
    mlp_d = nc.dram_tensor("mlp", [G, 24], BF16, kind="ExternalInput").ap()
    out_d = nc.dram_tensor("out", [JD * H, G, W], BF16, kind="ExternalOutput").ap()

    with TileContext(nc) as tc:
        with tc.tile_pool(name="const", bufs=1) as cpool, \
             tc.tile_pool(name="dtmp", bufs=1) as dtmpp, \
             tc.tile_pool(name="dt2", bufs=2) as dt2p, \
             tc.tile_pool(name="dtu2", bufs=2) as dtu2p, \
             tc.tile_pool(name="bc", bufs=2) as bcp, \
             tc.tile_pool(name="bb", bufs=2) as bbp, \
             tc.tile_pool(name="csm", bufs=1) as csmp, \
             tc.tile_pool(name="apool", bufs=int(os.environ.get("KERNEL_AB", "2"))) as apl, \
             tc.tile_pool(name="bpool", bufs=int(os.environ.get("KERNEL_BB", "2"))) as bpl, \
             tc.tile_pool(name="hpool", bufs=int(os.environ.get("KERNEL_HB", "4"))) as hpl, \
             tc.tile_pool(name="tpool", bufs=1) as tpl, \
             tc.tile_pool(name="cstg", bufs=1) as cstgp, \
             tc.tile_pool(name="epi", bufs=1) as epi, \
             tc.tile_pool(name="pproj", bufs=2, space="PSUM") as pproj, \
             tc.tile_pool(name="pz", bufs=1, space="PSUM") as pz, \
             tc.tile_pool(name="pc", bufs=1, space="PSUM") as pc:

            _ld = mybir.InstLoadActFuncSet(
                name=nc.get_next_instruction_name(), act_func_set_id=6,
                ins=[], outs=[])
            nc.scalar.add_instruction(_ld)
            wseL = cpool.tile([C, 576], BF16)
            nc.sync.dma_start(wseL[:], wse_d[0:C, :])
            wseR = cpool.tile([C, 576], BF16)
            nc.sync.dma_start(wseR[:], wse_d[C:2 * C, :])
            avec = cpool.tile([128, 8], F32)
            nc.sync.dma_start(avec[:], avec_d[:])
            featsb = cpool.tile([C, 2 * FROW], BF16)
            HB = PAD + HH * W
            nc.sync.dma_start(featsb[:, 0:HB], feat_d[:, 0:HB])
            nc.gpsimd.dma_start(featsb[:, FROW:FROW + HB],
                                feat_d[:, FROW:FROW + HB])
            nc.sync.dma_start(featsb[:, HB:FROW], feat_d[:, HB:FROW])
            nc.gpsimd.dma_start(featsb[:, FROW + HB:2 * FROW],
                                feat_d[:, FROW + HB:2 * FROW])
            wbf = cpool.tile([128, 48], BF16)
            nc.sync.dma_start(wbf[:], wbf_d[:])
            umask = cpool.tile([32, JD * W], BF16)
            nc.sync.dma_start(umask[:], umask_d[:])
            mnegs = cpool.tile([128, 2 * G * W], BF16)
            nc.sync.dma_start(mnegs[:], mnegs_d[:])
            invc = cpool.tile([G, JD], F32)
            nc.sync.dma_start(invc[:], invc_d[:])
            wsp = cpool.tile([128, 4], F32)
            nc.sync.dma_start(wsp[:], wsp_d[:])
            mlpw = cpool.tile([G, 24], BF16)
            nc.sync.dma_start(mlpw[:], mlp_d[:])
            z1ones = cpool.tile([5, 8], BF16)
            nc.sync.dma_start(z1ones[4:5, 0:6], mlpw[0:1, 16:22])

            def _one_iter():
              acc24 = epi.tile([G, 24], F32, tag="acc24")    # per-(chunk,s4) sums
              S1 = epi.tile([128, G * W], BF16, tag="S1")    # spatial rows 0-127
              S2 = epi.tile([64, G * W], BF16, tag="S2")     # spatial rows 128-191
              _btmask = int(os.environ.get("KERNEL_BTPOOL", "10"))
              _btlist = [int(x) for x in os.environ.get(
                  "KERNEL_BTLIST", "10,10,10,10,10,10").split(",")]

              def phase_s4(ch):
                  j, hh = divmod(ch, 2)
                  base = hh * CCOLS
                  dt2 = dt2p.tile([128, CCOLS], BF16)
                  dtu2 = dtu2p.tile([128, CCOLS], BF16)
                  bc = bcp.tile([32, CCOLS], BF16)
                  for s4 in range(4):
                      cs = base + s4 * 1024
                      sl = slice(s4 * 1024, s4 * 1024 + 1024)
                      ftL = featsb[:, PAD + cs: PAD + cs + 1024]
                      ftR = featsb[:, FROW + PAD + cs - j: FROW + PAD + cs - j + 1024]

                      pd = pproj.tile([128, 1024], F32, tag="proj")
                      for hv in range(2):
                          cv = slice(512 * hv, 512 * hv + 512)
                          nc.tensor.matmul(pd[:, cv], lhsT=wseL[:, 256:384],
                                           rhs=ftL[:, cv], start=True, stop=False)
                          nc.tensor.matmul(pd[:, cv], lhsT=wseR[:, 384:512],
                                           rhs=ftR[:, cv], start=False, stop=True)
                      dm = dtmpp.tile([128, 1024], BF16, tag="dm")
                      nc.scalar.activation(dm[:], pd[:], AF.Exp,
                                           bias=avec[:, 0:1], scale=1.0)
                      nc.scalar.activation(dt2[:, sl], dm[:], AF.Ln, bias=1.0,
                                           scale=1.0)

                      pu = pproj.tile([128, 1024], F32, tag="proj")
                      for hv in range(2):
                          cv = slice(512 * hv, 512 * hv + 512)
                          nc.tensor.matmul(pu[:, cv], lhsT=wseL[:, 0:128],
                                           rhs=ftL[:, cv], start=True, stop=False)
                          nc.tensor.matmul(pu[:, cv], lhsT=wseR[:, 128:256],
                                           rhs=ftR[:, cv], start=False, stop=True)
                      if ch == 0 and int(os.environ.get("KERNEL_W0", "1")):
                          nc.vector.tensor_tensor(dtu2[:, sl], dt2[:, sl],
                                                  pu[:], OP.mult)
                      else:
                          u_sb = dtmpp.tile([128, 1024], BF16, tag="usb")
                          nc.scalar.activation(u_sb[:], pu[:], AF.Copy,
                                               bias=0.0, scale=1.0)
                          nc.vector.tensor_tensor(dtu2[:, sl], dt2[:, sl],
                                                  u_sb[:], OP.mult)

                      pb = pproj.tile([128, 1024], F32, tag="proj")
                      for hv in range(2):
                          cv = slice(512 * hv, 512 * hv + 512)
                          nc.tensor.matmul(pb[0:32, cv], lhsT=wseL[:, 512:544],
                                           rhs=ftL[:, cv], start=True, stop=False)
                          nc.tensor.matmul(pb[0:32, cv], lhsT=wseR[:, 544:576],
                                           rhs=ftR[:, cv], start=False, stop=True)
                      mview = umask[:, j * W:(j + 1) * W].unsqueeze(1) \
                          .broadcast_to((32, 8, W))
                      nc.vector.scalar_tensor_tensor(
                          bc[:, sl].rearrange("p (a b) -> p a b", b=W),
                          pb[0:32, :].rearrange("p (a b) -> p a b", b=W), 1.0,
                          mview, OP.mult, OP.mult)
                  return dt2, dtu2, bc

              def phase_pairs(st, prange, htiles, csm=None, _btm=None):
                  dt2, dtu2, bc = st
                  if _btm is None:
                      _btm = _btmask
                  _bbq = nc.sync if int(os.environ.get("KERNEL_BBSP", "0")) else nc.gpsimd
                  if csm is None:
                      csm = csmp.tile([128, CCOLS], BF16)
                      _bbq.dma_start(
                          csm[:],
                          bc[16:24, :].unsqueeze(1).broadcast_to((8, 16, CCOLS)))
                  for p in prange:
                      bb = bbp.tile([128, CCOLS], BF16)
                      bt = bpl.tile([128, CCOLS], BF16)
                      _sliced = (ch == 0 and
                                 p < int(os.environ.get("KERNEL_WSLICE", "2")))
                      if _sliced:
                          for s4 in range(4):
                              sl = slice(s4 * 1024, s4 * 1024 + 1024)
                              _bbq.dma_start(
                                  bb[:, sl],
                                  bc[8 + 2 * p:8 + 2 * p + 2, sl].unsqueeze(1)
                                  .broadcast_to((2, 64, 1024)))
                      else:
                          _bbq.dma_start(
                              bb[:],
                              bc[8 + 2 * p:8 + 2 * p + 2, :].unsqueeze(1)
                              .broadcast_to((2, 64, CCOLS)))
                      av = apl.tile([128, CCOLS], BF16)
                      nc.scalar.activation(av[:], dt2[:], AF.Exp,
                                           bias=0.0, scale=avec[:, 1 + p: 2 + p])
                      nc.vector.memset(
                          av[:].rearrange("p (h w) -> p h w", w=W)[:, :, 0:1], 0)
                      bteng = nc.gpsimd if (_btm >> p) & 1 else nc.vector
                      if _sliced:
                          for s4 in range(4):
                              sl = slice(s4 * 1024, s4 * 1024 + 1024)
                              bteng.tensor_tensor(bt[:, sl], dtu2[:, sl],
                                                  bb[:, sl], OP.mult)
                      else:
                          bteng.tensor_tensor(bt[:], dtu2[:], bb[:], OP.mult)
                      hT = hpl.tile([128, CCOLS], BF16)
                      nc.vector.tensor_tensor_scan(hT[:], av[:], bt[:], 0.0,
                                                   OP.mult, OP.add)
                      htiles.append(hT)
                  return csm

              def phase_tail(ch, st, pr):
                  j, hh = divmod(ch, 2)
                  dt2, dtu2, bc = st
                  htiles, csm = pr
                  tt = tpl.tile([128, CCOLS], BF16, tag="tt")
                  for s8 in range(4):
                      sl10 = slice(s8 * 1024, s8 * 1024 + 1024)
                      zp = pz.tile([128, 1024], F32, tag="zp")
                      for half in range(2):
                          zv = slice(512 * half, 512 * half + 512)
                          sl5 = slice(s8 * 1024 + 512 * half,
                                      s8 * 1024 + 512 * half + 512)
                          for p in range(4):
                              nc.tensor.matmul(zp[32 * p:32 * p + 32, zv],
                                               lhsT=wbf[:, 0:32],
                                               rhs=htiles[p][:, sl5],
                                               start=True, stop=True,
                                               tile_position=(0, 32 * p))
                      _tta = int(os.environ.get("KERNEL_TTACT", "4"))
                      if ch == NCH - 1 and int(os.environ.get("KERNEL_T5", "1")):
                          _tta = 0
                      if s8 < _tta:
                          z_sb = epi.tile([128, 1024], BF16, tag="scr4k")
                          nc.scalar.activation(z_sb[:], zp[:], AF.Copy,
                                               bias=0.0, scale=1.0)
                          nc.vector.tensor_tensor(tt[:, sl10], z_sb[:],
                                                  csm[:, sl10], OP.mult)
                      else:
                          nc.vector.scalar_tensor_tensor(tt[:, sl10], zp[:], 1.0,
                                                         csm[:, sl10], OP.mult, OP.mult)

                  if int(os.environ.get("KERNEL_CSHARE", "0")):
                      cstg = tpl.tile([8, CCOLS], BF16, tag="tt")
                  else:
                      cstg = cstgp.tile([8, CCOLS], BF16)
                  for s4 in range(4):
                      sl = slice(s4 * 1024, s4 * 1024 + 1024)
                      cp = pc.tile([8, 1024], F32, tag="cp")
                      for hv in range(2):
                          cv = slice(512 * hv, 512 * hv + 512)
                          cg = slice(s4 * 1024 + 512 * hv, s4 * 1024 + 512 * hv + 512)
                          nc.tensor.matmul(cp[:, cv], lhsT=wbf[:, 32:40],
                                           rhs=tt[:, cg], start=True, stop=False)
                          nc.tensor.matmul(cp[:, cv], lhsT=wbf[0:8, 40:48],
                                           rhs=bc[0:8, cg], start=False, stop=True)
                      nc.scalar.activation(
                          cstg[:, sl], cp[:], AF.Copy, bias=0.0, scale=1.0,
                          accum_out=acc24[:, ch * 4 + s4: ch * 4 + s4 + 1])

                  row0 = j * 64 + hh * 32
                  st_t, st_r = (S1, row0) if row0 < 128 else (S2, row0 - 128)
                  _sgs = int(os.environ.get("KERNEL_SGSPLIT", "0")) or \
                      (ch == NCH - 1 and int(os.environ.get("KERNEL_SGLAST", "1")))
                  for g in range(G):
                      q = nc.gpsimd if (_sgs and g % 2) else nc.sync
                      q.dma_start(
                          st_t[st_r:st_r + 32, g * W:(g + 1) * W],
                          cstg[g:g + 1, :].rearrange("p (h w) -> p h w", w=W))

              from concourse import bass_isa
              rr = epi.tile([64, JD * G], BF16, tag="rr")
              ppool = epi.tile([G, 8], BF16, tag="ppool")
              arr = epi.tile([64, JD * G], BF16, tag="arr")

              def mx_path_s1():
                  sm1 = epi.tile([128, G * W], BF16, tag="sm1")
                  nc.vector.tensor_tensor(sm1[:], S1[:], mnegs[:, 0:G * W], OP.add)
                  r1 = epi.tile([128, G], BF16, tag="r1")
                  nc.vector.tensor_reduce(
                      r1[:], sm1[:].rearrange("p (g w) -> p g w", w=W),
                      AX.X, OP.max)
                  nc.gpsimd.dma_start(rr[:, 0:G], r1[0:64, :])
                  nc.sync.dma_start(rr[:, G:2 * G], r1[64:128, :])
                  nc.gpsimd.partition_all_reduce(
                      arr[:, 0:2 * G], rr[:, 0:2 * G], 64, bass_isa.ReduceOp.max)
                  nc.gpsimd.dma_start(ppool[:, 3:4], arr[0:1, 0:G])
                  nc.sync.dma_start(ppool[:, 4:5], arr[0:1, G:2 * G])

              def mx_path_s2():
                  sm2 = epi.tile([64, G * W], BF16, tag="sm2")
                  nc.vector.tensor_tensor(sm2[:], S2[:],
                                          mnegs[0:64, G * W:2 * G * W], OP.add)
                  r2 = epi.tile([64, G], BF16, tag="r2")
                  nc.vector.tensor_reduce(
                      r2[:], sm2[:].rearrange("p (g w) -> p g w", w=W),
                      AX.X, OP.max)
                  nc.gpsimd.partition_all_reduce(
                      arr[:, 2 * G:3 * G], r2[:], 64, bass_isa.ReduceOp.max)
                  # transpose [1,8] -> [8,1] on PE (outer product with 1.0)
                  # instead of a ~2.5us transposing DMA
                  pmx = pc.tile([8, 1], F32, tag="cp")
                  nc.tensor.matmul(pmx[:], lhsT=arr[0:1, 2 * G:3 * G],
                                   rhs=mlpw[0:1, 16:17], start=True, stop=True)
                  nc.vector.tensor_copy(ppool[:, 7:8], pmx[:])

              gb1 = epi.tile([128, 8], BF16, tag="gb1")
              gb2 = epi.tile([64, 8], BF16, tag="gb2")

              def epi01():
                  # channel attention for j0/j1: their chunks (0-3) are done
                  avgr01 = epi.tile([G, 2], F32, tag="avgr01")
                  nc.vector.tensor_reduce(
                      avgr01[:], acc24[:, 0:16].rearrange("p (j r) -> p j r", r=8),
                      AX.X, OP.add)
                  nc.vector.tensor_tensor(ppool[:, 0:2], avgr01[:],
                                          invc[:, 0:2], OP.mult)
                  z1p01 = pc.tile([4, 4], F32, tag="cp")
                  nc.tensor.matmul(z1p01[:, 0:2], lhsT=mlpw[:, 0:4],
                                   rhs=ppool[:, 0:2], start=True, stop=True)
                  nc.tensor.matmul(z1p01[:, 2:4], lhsT=mlpw[:, 0:4],
                                   rhs=ppool[:, 3:5], start=True, stop=True)
                  nc.scalar.activation(z1ones[0:4, 0:2], z1p01[:, 0:2], AF.Relu,
                                       bias=mlpw[0:4, 12:13], scale=1.0)
                  nc.scalar.activation(z1ones[0:4, 3:5], z1p01[:, 2:4], AF.Relu,
                                       bias=mlpw[0:4, 12:13], scale=1.0)
                  gp01 = pc.tile([2, 8], F32, tag="cp")
                  nc.tensor.matmul(gp01[:], lhsT=z1ones[0:5, 0:2],
                                   rhs=mlpw[0:5, 4:12], start=True, stop=False)
                  nc.tensor.matmul(gp01[:], lhsT=z1ones[0:5, 3:5],
                                   rhs=mlpw[0:5, 4:12], start=False, stop=True)
                  # sigmoid via exp + reciprocal: stays on act table 6
                  eg01 = epi.tile([2, 8], F32, tag="r1")
                  nc.scalar.activation(eg01[:], gp01[:], AF.Exp,
                                       bias=0.0, scale=-1.0)
                  egp01 = epi.tile([2, 8], F32, tag="r2")
                  nc.vector.tensor_scalar_add(egp01[:], eg01[:], 1.0)
                  chg01 = epi.tile([2, 8], BF16, tag="chg01")
                  with nc.allow_low_precision(reason="bf16 channel gate"):
                      nc.vector.reciprocal(chg01[:], egp01[:])
                  for jj, q in ((0, nc.gpsimd), (1, nc.sync)):
                      q.dma_start(
                          gb1[64 * jj:64 * jj + 64, :],
                          chg01[jj:jj + 1, :].unsqueeze(1)
                          .broadcast_to((1, 64, 8)))

              def s_gate(Sg, gb, rows, obase, tg):
                  Sgf = epi.tile([rows, G * W], BF16,
                                 tag="sm1" if rows == 128 else "sm2")
                  gview = gb[0:rows, :].unsqueeze(2).broadcast_to((rows, G, W))
                  nc.vector.tensor_tensor(
                      Sgf[:].rearrange("p (a b) -> p a b", b=W),
                      Sg[:].rearrange("p (a b) -> p a b", b=W), gview, OP.mult)
                  sv = Sgf[:].rearrange("p (g w) -> p w g", g=G)
                  ssum = epi.tile([rows, W], BF16, tag="ss" + tg)
                  with nc.allow_low_precision(reason="8-term channel mean for gate"):
                      nc.vector.tensor_reduce(ssum[:], sv, AX.X, OP.add)
                  smx = epi.tile([rows, W], BF16, tag="sm" + tg)
                  nc.vector.tensor_reduce(smx[:], sv, AX.X, OP.max)
                  q1 = epi.tile([rows, W], BF16, tag="q1" + tg)
                  nc.vector.tensor_scalar_mul(q1[:], smx[:], wsp[0:rows, 1:2])
                  gi = epi.tile([rows, W], BF16, tag="gi" + tg)
                  nc.vector.scalar_tensor_tensor(gi[:], ssum[:], wsp[0:rows, 0:1],
                                                 q1[:], OP.mult, OP.add)
                  # sigmoid(gi + b_sp) = 1/(1 + e^(-gi - b_sp)); wsp col3 = -b_sp
                  ei = epi.tile([rows, W], BF16, tag="sm" + tg)
                  nc.scalar.activation(ei[:], gi[:], AF.Exp,
                                       bias=wsp[0:rows, 3:4], scale=-1.0)
                  e1 = epi.tile([rows, W], BF16, tag="q1" + tg)
                  nc.vector.tensor_scalar_add(e1[:], ei[:], 1.0)
                  sg = epi.tile([rows, W], BF16, tag="gi" + tg)
                  with nc.allow_low_precision(reason="bf16 spatial gate"):
                      nc.vector.reciprocal(sg[:], e1[:])
                  O = epi.tile([rows, G * W], BF16, tag="scr4k")
                  oview = sg[:].unsqueeze(1).broadcast_to((rows, G, W))
                  nc.vector.tensor_tensor(
                      O[:].rearrange("p (a b) -> p a b", b=W),
                      Sgf[:].rearrange("p (a b) -> p a b", b=W), oview, OP.mult)
                  nc.gpsimd.dma_start(
                      out_d[obase:obase + rows].rearrange("r g w -> r (g w)"), O[:])

              # ---- software-pipelined chunk schedule ----
              # emission order: s4(k) | tail(k-1) | pairs01(k) | s4(k+1) |
              #                 pairs23(k) | tail(k) | ...
              prev = None
              pend = None       # (ch, st, htiles, csm) with pairs23 pending
              for ch in range(NCH):
                  st = phase_s4(ch)
                  if pend is not None:
                      phase_pairs(pend[1], range(2, 4), pend[2], pend[3],
                                  _btm=_btlist[pend[0]])
                      prev = (pend[0], pend[1], (pend[2], pend[3]))
                      pend = None
                  if prev is not None:
                      phase_tail(prev[0], prev[1], prev[2])
                      if prev[0] == 3:
                          mx_path_s1()
                          epi01()
                      if prev[0] == 4:
                          s_gate(S1, gb1, 128, 0, "a")
                      prev = None
                  htiles = []
                  csm = phase_pairs(st, range(0, 2), htiles, _btm=_btlist[ch])
                  pend = (ch, st, htiles, csm)
              phase_pairs(pend[1], range(2, 4), pend[2], pend[3],
                          _btm=_btlist[pend[0]])
              phase_tail(pend[0], pend[1], (pend[2], pend[3]))
              mx_path_s2()

              # ---------- j2 epilogue (only piece left after last tail) ----
              avgr2 = epi.tile([G, 1], F32, tag="avgr2")
              nc.vector.tensor_reduce(avgr2[:], acc24[:, 16:24], AX.X, OP.add)
              nc.vector.tensor_tensor(ppool[:, 6:7], avgr2[:], invc[:, 2:3],
                                      OP.mult)
              z1p2 = pc.tile([4, 2], F32, tag="cp")
              nc.tensor.matmul(z1p2[:], lhsT=mlpw[:, 0:4], rhs=ppool[:, 6:8],
                               start=True, stop=True)
              nc.scalar.activation(z1ones[0:4, 2:3], z1p2[:, 0:1], AF.Relu,
                                   bias=mlpw[0:4, 12:13], scale=1.0)
              nc.scalar.activation(z1ones[0:4, 5:6], z1p2[:, 1:2], AF.Relu,
                                   bias=mlpw[0:4, 12:13], scale=1.0)
              gp2 = pc.tile([1, 8], F32, tag="cp")
              nc.tensor.matmul(gp2[:], lhsT=z1ones[0:5, 2:3], rhs=mlpw[0:5, 4:12],
                               start=True, stop=False)
              nc.tensor.matmul(gp2[:], lhsT=z1ones[0:5, 5:6], rhs=mlpw[0:5, 4:12],
                               start=False, stop=True)
              eg2 = epi.tile([1, 8], F32, tag="r1")
              nc.scalar.activation(eg2[:], gp2[:], AF.Exp, bias=0.0, scale=-1.0)
              egp2 = epi.tile([1, 8], F32, tag="avgr01")
              nc.vector.tensor_scalar_add(egp2[:], eg2[:], 1.0)
              chg2 = epi.tile([1, 8], BF16, tag="chg2")
              with nc.allow_low_precision(reason="bf16 channel gate"):
                  nc.vector.reciprocal(chg2[:], egp2[:])
              nc.gpsimd.partition_broadcast(gb2[:], chg2[:], channels=64)
              s_gate(S2, gb2, 64, 128, "b")
            for _it in range(_ITERS):
                _one_iter()

    nc.compile()
    return nc


def _host_inputs(inputs):
    """Build the 8 per-core input maps from the full problem inputs."""
    import ml_dtypes
    L = _f32(inputs["featuresL"])[0]          # [C,H,W]
    R = _f32(inputs["featuresR"])[0]
    W_in = _f32(inputs["W_in"])
    W_dt = _f32(inputs["W_dt"])
    b_dt = _f32(inputs["b_dt"])
    W_B = _f32(inputs["W_B"])
    W_C = _f32(inputs["W_C"])
    A = -np.exp(_f32(inputs["A_log"]))        # [E,S]
    D_skip = _f32(inputs["D_skip"])
    W_out = _f32(inputs["W_out"])
    W1, b1 = _f32(inputs["W1"]), _f32(inputs["b1"])
    W2, b2 = _f32(inputs["W2"]), _f32(inputs["b2"])
    w_sp, b_sp = _f32(inputs["w_sp"]), _f32(inputs["b_sp"])

    # stationary weights [64, 576]
    idx = np.arange(128) % 64
    wse = np.zeros((2 * C, 576), np.float32)
    wse[0:32, 0:128] = W_in[0::2][:, idx]
    wse[32:64, 128:256] = W_in[1::2][:, idx]
    wse[0:32, 256:384] = W_dt[0::2][:, idx]
    wse[32:64, 384:512] = W_dt[1::2][:, idx]
    W_comb = W_in @ (D_skip[:, None] * W_out)        # [64(c), G]
    wse[0:32, 512:520] = W_comb[0::2]
    wse[0:32, 520:528] = W_B[0::2]
    wse[0:32, 528:536] = W_C[0::2]
    wse[32:64, 544:552] = W_comb[1::2]
    wse[32:64, 552:560] = W_B[1::2]
    wse[32:64, 560:568] = W_C[1::2]

    # bf16 stationaries [128, 48]
    wbf = np.zeros((128, 48), np.float32)
    for row in range(128):
        cc, e = divmod(row, 64)
        for q in range(32):
            c2, g = q // 16, q % 16
            if g < 8 and cc == c2:
                wbf[row, q] = W_out[e, g]
    for p4 in range(4):
        for local in range(32):
            c2, g = local // 16, local % 16
            if g < 8:
                wbf[32 * p4 + local, 32 + g] = 1.0
    wbf[0:8, 40:48] = np.eye(8, dtype=np.float32)

    avec = np.zeros((128, 8), np.float32)
    avec[:, 0] = b_dt[idx]
    for p4 in range(4):
        cc = np.arange(128) // 64
        avec[:, 1 + p4] = A[idx, 2 * p4 + cc]

    wspv = np.zeros((128, 4), np.float32)
    wspv[:, 0] = w_sp[0] / G
    wspv[:, 1] = w_sp[1]
    wspv[:, 2] = np.float32(np.asarray(b_sp).reshape(-1)[0]) if np.asarray(b_sp).size else 0.0
    wspv[:, 3] = -wspv[:, 2]

    mlpv = np.zeros((G, 24), np.float32)
    mlpv[:, 0:4] = W1
    mlpv[0:4, 4:12] = W2
    mlpv[4, 4:12] = 2.0 * b2
    mlpv[0:4, 12] = b1
    mlpv[0, 16:19] = 1.0

    maps = []
    wi = np.arange(W)
    for k in range(NCORES):
        d0 = JD * k
        Rsh = np.zeros_like(R)
        if d0 > 0:
            Rsh[:, :, d0:] = R[:, :, :-d0]
        else:
            Rsh = R
        feat = np.zeros((C, 2 * FROW), np.float32)
        feat[:, PAD:PAD + HW] = L.reshape(C, HW)
        feat[:, FROW + PAD:] = Rsh.reshape(C, HW)

        umask = np.zeros((32, JD * W), np.float32)
        for j in range(JD):
            umask[:, j * W:(j + 1) * W] = (wi >= d0 + j).astype(np.float32)[None]

        # S-layout max-pool masks: S1 rows (j=0,1), S2 rows (j=2)
        mnegs = np.full((128, 2 * G * W), -1e30, np.float32)
        for j in range(2):
            row_mask = np.where(wi >= d0 + j, 0.0, -1e30)          # [W]
            mnegs[j * 64:(j + 1) * 64, 0:G * W] = np.tile(row_mask, G)[None]
        mnegs[0:64, G * W:2 * G * W] = np.tile(
            np.where(wi >= d0 + 2, 0.0, -1e30), G)[None]

        invc = np.zeros((G, JD), np.float32)
        for j in range(JD):
            invc[:, j] = 1.0 / (H * (W - (d0 + j)))

        maps.append({
            "feat": feat.astype(ml_dtypes.bfloat16),
            "wse": wse.astype(ml_dtypes.bfloat16),
            "wbf": wbf.astype(ml_dtypes.bfloat16),
            "avec": avec,
            "umask": umask.astype(ml_dtypes.bfloat16),
            "mnegs": mnegs.astype(ml_dtypes.bfloat16),
            "invc": invc,
            "wsp": wspv,
            "mlp": mlpv.astype(ml_dtypes.bfloat16),
        })
    return maps


def kernel(**inputs):
    from concourse.bass_utils import run_bass_kernel_spmd

    if "nc" not in _compiled:
        _compiled["nc"] = _build_program()
    nc = _compiled["nc"]

    maps = _host_inputs(inputs)
    res = run_bass_kernel_spmd(nc, maps, list(range(NCORES))).results

    vol = np.zeros((1, G, DV, H, W), np.float32)
    for k in range(NCORES):
        o = np.asarray(res[k]["out"], np.float32).reshape(JD, H, G, W)        # [j,h,g,w]
        vol[0, :, JD * k:JD * k + JD] = np.transpose(o, (2, 0, 1, 3))
    return vol



# revision 40
# speedup vs baseline: 1.0020x; 1.0020x over previous
"""Trainium2 Bass kernel for nn_BuildCostVolume (stereo cost volume + Mamba scan).

Sharding: disparity axis (24) split as 3 per core across 8 cores; core k
handles disparities d = 3k+j (j in 0..2, compile-time; host pre-shifts
featuresR by 3k so the SPMD program is identical across cores).

Per-core pipeline (software-pipelined across the 6 (j, h-half) chunks as
s4(k) | tail(k-1) | pairs01(k) | s4(k+1) | pairs23(k) | tail(k) | ...):
  - Features loaded once as bf16; u/dt/B/C/D projections on PE from L and
    shifted-R views with even/odd split weights (channel interleave trick).
  - dt = softplus via Exp + Ln(x+1) on ACT; u evicted via ACT Copy so the
    dt*u multiply runs at the DVE 2x (16-bit) rate.
  - Decay a = exp(A*dt) via ACT per-partition scale in an (s-pair x e)
    128-partition layout; B broadcast via SBUF-to-SBUF DMA (Pool queue);
    b = dt*u*B on DVE, with pairs 1,3 offloaded to GPSIMD to balance.
  - Mamba recurrence h = a*h + b via DVE tensor_tensor_scan over flattened
    (row, w) with a[w=0]=0 so each image row restarts the scan.
  - y/cost contraction on PE (block-diag W_out fold, C multiply at PSUM
    eviction, partition-sum + D-term matmul); cstg evicted on ACT with
    fused avg-pool accumulation.
  - Channel-attn max pool from the spatial S layout: masked add + per-g
    max on 128 partitions, GPSIMD partition_all_reduce, tiny transposing
    DMAs; MLP in bf16; spatial attention as in the reference.
  - Output written [j*64+h, g, w] bf16 and transposed/cast on host.
"""
import os
import numpy as np

C, H, W, DV = 32, 64, 128, 24
_NCH_ENV = int(os.environ.get("KERNEL_NCH", "6"))
_SKIP_EPI = bool(int(os.environ.get("KERNEL_SKIP_EPI", "0")))
_SKIP_PAIRS = bool(int(os.environ.get("KERNEL_SKIP_PAIRS", "0")))
_ITERS = int(os.environ.get("KERNEL_ITERS", "1"))
E, S, G = 64, 8, 8
NCORES, JD = 8, 3          # cores, disparities per core
PAD = 8                    # leading zero columns in feature tensors
HH = 32                    # h rows per chunk
NCH = 6                    # chunks = (j, h-half)
CCOLS = HH * W             # 4096 columns per chunk
HW = H * W                 # 8192
FROW = PAD + HW            # 8200 cols per feature image

_compiled = {}


def _f32(x):
    return np.ascontiguousarray(np.asarray(x, np.float32))


def _build_program():
    import concourse.bacc as bacc
    import concourse.mybir as mybir
    from concourse.tile import TileContext

    F32 = mybir.dt.float32
    BF16 = mybir.dt.bfloat16
    AF = mybir.ActivationFunctionType
    AX = mybir.AxisListType
    OP = mybir.AluOpType

    nc = bacc.Bacc("TRN2", target_bir_lowering=False, debug=False,
                   num_devices=NCORES)

    feat_d = nc.dram_tensor("feat", [C, 2 * FROW], BF16, kind="ExternalInput").ap()
    wse_d = nc.dram_tensor("wse", [2 * C, 576], BF16, kind="ExternalInput").ap()
    wbf_d = nc.dram_tensor("wbf", [128, 48], BF16, kind="ExternalInput").ap()
    avec_d = nc.dram_tensor("avec", [128, 8], F32, kind="ExternalInput").ap()
    umask_d = nc.dram_tensor("umask", [32, JD * W], BF16, kind="ExternalInput").ap()
    mnegs_d = nc.dram_tensor("mnegs", [128, 2 * G * W], BF16, kind="ExternalInput").ap()
    invc_d = nc.dram_tensor("invc", [G, JD], F32, kind="ExternalInput").ap()
    wsp_d = nc.dram_tensor("wsp", [128, 4], F32, kind="ExternalInput").ap()
    mlp_d = nc.dram_tensor("mlp", [G, 24], BF16, kind="ExternalInput").ap()
    out_d = nc.dram_tensor("out", [JD * H, G, W], BF16, kind="ExternalOutput").ap()

    with TileContext(nc) as tc:
        with tc.tile_pool(name="const", bufs=1) as cpool, \
             tc.tile_pool(name="dtmp", bufs=1) as dtmpp, \
             tc.tile_pool(name="dt2", bufs=2) as dt2p, \
             tc.tile_pool(name="dtu2", bufs=2) as dtu2p, \
             tc.tile_pool(name="bc", bufs=2) as bcp, \
             tc.tile_pool(name="bb", bufs=2) as bbp, \
             tc.tile_pool(name="csm", bufs=1) as csmp, \
             tc.tile_pool(name="apool", bufs=int(os.environ.get("KERNEL_AB", "2"))) as apl, \
             tc.tile_pool(name="bpool", bufs=int(os.environ.get("KERNEL_BB", "2"))) as bpl, \
             tc.tile_pool(name="hpool", bufs=int(os.environ.get("KERNEL_HB", "4"))) as hpl, \
             tc.tile_pool(name="tpool", bufs=1) as tpl, \
             tc.tile_pool(name="cstg", bufs=1) as cstgp, \
             tc.tile_pool(name="epi", bufs=1) as epi, \
             tc.tile_pool(name="pproj", bufs=2, space="PSUM") as pproj, \
             tc.tile_pool(name="pz", bufs=1, space="PSUM") as pz, \
             tc.tile_pool(name="pc", bufs=1, space="PSUM") as pc:

            _ld = mybir.InstLoadActFuncSet(
                name=nc.get_next_instruction_name(), act_func_set_id=6,
                ins=[], outs=[])
            nc.scalar.add_instruction(_ld)
            wseL = cpool.tile([C, 576], BF16)
            nc.sync.dma_start(wseL[:], wse_d[0:C, :])
            wseR = cpool.tile([C, 576], BF16)
            nc.sync.dma_start(wseR[:], wse_d[C:2 * C, :])
            avec = cpool.tile([128, 8], F32)
            nc.sync.dma_start(avec[:], avec_d[:])
            featsb = cpool.tile([C, 2 * FROW], BF16)
            HB = PAD + HH * W
            nc.sync.dma_start(featsb[:, 0:HB], feat_d[:, 0:HB])
            nc.gpsimd.dma_start(featsb[:, FROW:FROW + HB],
                                feat_d[:, FROW:FROW + HB])
            nc.sync.dma_start(featsb[:, HB:FROW], feat_d[:, HB:FROW])
            nc.gpsimd.dma_start(featsb[:, FROW + HB:2 * FROW],
                                feat_d[:, FROW + HB:2 * FROW])
            wbf = cpool.tile([128, 48], BF16)
            nc.sync.dma_start(wbf[:], wbf_d[:])
            umask = cpool.tile([32, JD * W], BF16)
            nc.sync.dma_start(umask[:], umask_d[:])
            mnegs = cpool.tile([128, 2 * G * W], BF16)
            nc.sync.dma_start(mnegs[:], mnegs_d[:])
            invc = cpool.tile([G, JD], F32)
            nc.sync.dma_start(invc[:], invc_d[:])
            wsp = cpool.tile([128, 4], F32)
            nc.sync.dma_start(wsp[:], wsp_d[:])
            mlpw = cpool.tile([G, 24], BF16)
            nc.sync.dma_start(mlpw[:], mlp_d[:])
            z1ones = cpool.tile([5, 8], BF16)
            nc.sync.dma_start(z1ones[4:5, 0:6], mlpw[0:1, 16:22])

            def _one_iter():
              acc24 = epi.tile([G, 24], F32, tag="acc24")    # per-(chunk,s4) sums
              S1 = epi.tile([128, G * W], BF16, tag="S1")    # spatial rows 0-127
              S2 = epi.tile([64, G * W], BF16, tag="S2")     # spatial rows 128-191
              _btmask = int(os.environ.get("KERNEL_BTPOOL", "10"))
              _btlist = [int(x) for x in os.environ.get(
                  "KERNEL_BTLIST", "10,10,10,10,10,10").split(",")]

              def phase_s4(ch):
                  j, hh = divmod(ch, 2)
                  base = hh * CCOLS
                  dt2 = dt2p.tile([128, CCOLS], BF16)
                  dtu2 = dtu2p.tile([128, CCOLS], BF16)
                  bc = bcp.tile([32, CCOLS], BF16)
                  for s4 in range(4):
                      cs = base + s4 * 1024
                      sl = slice(s4 * 1024, s4 * 1024 + 1024)
                      ftL = featsb[:, PAD + cs: PAD + cs + 1024]
                      ftR = featsb[:, FROW + PAD + cs - j: FROW + PAD + cs - j + 1024]

                      pd = pproj.tile([128, 1024], F32, tag="proj")
                      for hv in range(2):
                          cv = slice(512 * hv, 512 * hv + 512)
                          nc.tensor.matmul(pd[:, cv], lhsT=wseL[:, 256:384],
                                           rhs=ftL[:, cv], start=True, stop=False)
                          nc.tensor.matmul(pd[:, cv], lhsT=wseR[:, 384:512],
                                           rhs=ftR[:, cv], start=False, stop=True)
                      dm = dtmpp.tile([128, 1024], BF16, tag="dm")
                      nc.scalar.activation(dm[:], pd[:], AF.Exp,
                                           bias=avec[:, 0:1], scale=1.0)
                      nc.scalar.activation(dt2[:, sl], dm[:], AF.Ln, bias=1.0,
                                           scale=1.0)

                      pu = pproj.tile([128, 1024], F32, tag="proj")
                      for hv in range(2):
                          cv = slice(512 * hv, 512 * hv + 512)
                          nc.tensor.matmul(pu[:, cv], lhsT=wseL[:, 0:128],
                                           rhs=ftL[:, cv], start=True, stop=False)
                          nc.tensor.matmul(pu[:, cv], lhsT=wseR[:, 128:256],
                                           rhs=ftR[:, cv], start=False, stop=True)
                      if ch == 0 and int(os.environ.get("KERNEL_W0", "1")):
                          nc.vector.tensor_tensor(dtu2[:, sl], dt2[:, sl],
                                                  pu[:], OP.mult)
                      else:
                          u_sb = dtmpp.tile([128, 1024], BF16, tag="usb")
                          nc.scalar.activation(u_sb[:], pu[:], AF.Copy,
                                               bias=0.0, scale=1.0)
                          nc.vector.tensor_tensor(dtu2[:, sl], dt2[:, sl],
                                                  u_sb[:], OP.mult)

                      pb = pproj.tile([128, 1024], F32, tag="proj")
                      for hv in range(2):
                          cv = slice(512 * hv, 512 * hv + 512)
                          nc.tensor.matmul(pb[0:32, cv], lhsT=wseL[:, 512:544],
                                           rhs=ftL[:, cv], start=True, stop=False)
                          nc.tensor.matmul(pb[0:32, cv], lhsT=wseR[:, 544:576],
                                           rhs=ftR[:, cv], start=False, stop=True)
                      mview = umask[:, j * W:(j + 1) * W].unsqueeze(1) \
                          .broadcast_to((32, 8, W))
                      nc.vector.scalar_tensor_tensor(
                          bc[:, sl].rearrange("p (a b) -> p a b", b=W),
                          pb[0:32, :].rearrange("p (a b) -> p a b", b=W), 1.0,
                          mview, OP.mult, OP.mult)
                  return dt2, dtu2, bc

              def phase_pairs(st, prange, htiles, csm=None, _btm=None):
                  dt2, dtu2, bc = st
                  if _btm is None:
                      _btm = _btmask
                  _bbq = nc.sync if int(os.environ.get("KERNEL_BBSP", "0")) else nc.gpsimd
                  if csm is None:
                      csm = csmp.tile([128, CCOLS], BF16)
                      _bbq.dma_start(
                          csm[:],
                          bc[16:24, :].unsqueeze(1).broadcast_to((8, 16, CCOLS)))
                  for p in prange:
                      bb = bbp.tile([128, CCOLS], BF16)
                      bt = bpl.tile([128, CCOLS], BF16)
                      _sliced = (ch == 0 and
                                 p < int(os.environ.get("KERNEL_WSLICE", "2")))
                      if _sliced:
                          for s4 in range(4):
                              sl = slice(s4 * 1024, s4 * 1024 + 1024)
                              _bbq.dma_start(
                                  bb[:, sl],
                                  bc[8 + 2 * p:8 + 2 * p + 2, sl].unsqueeze(1)
                                  .broadcast_to((2, 64, 1024)))
                      else:
                          _bbq.dma_start(
                              bb[:],
                              bc[8 + 2 * p:8 + 2 * p + 2, :].unsqueeze(1)
                              .broadcast_to((2, 64, CCOLS)))
                      av = apl.tile([128, CCOLS], BF16)
                      nc.scalar.activation(av[:], dt2[:], AF.Exp,
                                           bias=0.0, scale=avec[:, 1 + p: 2 + p])
                      nc.vector.memset(
                          av[:].rearrange("p (h w) -> p h w", w=W)[:, :, 0:1], 0)
                      bteng = nc.gpsimd if (_btm >> p) & 1 else nc.vector
                      if _sliced:
                          for s4 in range(4):
                              sl = slice(s4 * 1024, s4 * 1024 + 1024)
                              bteng.tensor_tensor(bt[:, sl], dtu2[:, sl],
                                                  bb[:, sl], OP.mult)
                      else:
                          bteng.tensor_tensor(bt[:], dtu2[:], bb[:], OP.mult)
                      hT = hpl.tile([128, CCOLS], BF16)
                      nc.vector.tensor_tensor_scan(hT[:], av[:], bt[:], 0.0,
                                                   OP.mult, OP.add)
                      htiles.append(hT)
                  return csm

              def phase_tail(ch, st, pr):
                  j, hh = divmod(ch, 2)
                  dt2, dtu2, bc = st
                  htiles, csm = pr
                  tt = tpl.tile([128, CCOLS], BF16, tag="tt")
                  for s8 in range(4):
                      sl10 = slice(s8 * 1024, s8 * 1024 + 1024)
                      zp = pz.tile([128, 1024], F32, tag="zp")
                      for half in range(2):
                          zv = slice(512 * half, 512 * half + 512)
                          sl5 = slice(s8 * 1024 + 512 * half,
                                      s8 * 1024 + 512 * half + 512)
                          for p in range(4):
                              nc.tensor.matmul(zp[32 * p:32 * p + 32, zv],
                                               lhsT=wbf[:, 0:32],
                                               rhs=htiles[p][:, sl5],
                                               start=True, stop=True,
                                               tile_position=(0, 32 * p))
                      _tta = int(os.environ.get("KERNEL_TTACT", "4"))
                      if ch == NCH - 1 and int(os.environ.get("KERNEL_T5", "1")):
                          _tta = 0
                      if s8 < _tta:
                          z_sb = epi.tile([128, 1024], BF16, tag="scr4k")
                          nc.scalar.activation(z_sb[:], zp[:], AF.Copy,
                                               bias=0.0, scale=1.0)
                          nc.vector.tensor_tensor(tt[:, sl10], z_sb[:],
                                                  csm[:, sl10], OP.mult)
                      else:
                          nc.vector.scalar_tensor_tensor(tt[:, sl10], zp[:], 1.0,
                                                         csm[:, sl10], OP.mult, OP.mult)

                  if int(os.environ.get("KERNEL_CSHARE", "0")):
                      cstg = tpl.tile([8, CCOLS], BF16, tag="tt")
                  else:
                      cstg = cstgp.tile([8, CCOLS], BF16)
                  for s4 in range(4):
                      sl = slice(s4 * 1024, s4 * 1024 + 1024)
                      cp = pc.tile([8, 1024], F32, tag="cp")
                      for hv in range(2):
                          cv = slice(512 * hv, 512 * hv + 512)
                          cg = slice(s4 * 1024 + 512 * hv, s4 * 1024 + 512 * hv + 512)
                          nc.tensor.matmul(cp[:, cv], lhsT=wbf[:, 32:40],
                                           rhs=tt[:, cg], start=True, stop=False)
                          nc.tensor.matmul(cp[:, cv], lhsT=wbf[0:8, 40:48],
                                           rhs=bc[0:8, cg], start=False, stop=True)
                      nc.scalar.activation(
                          cstg[:, sl], cp[:], AF.Copy, bias=0.0, scale=1.0,
                          accum_out=acc24[:, ch * 4 + s4: ch * 4 + s4 + 1])

                  row0 = j * 64 + hh * 32
                  st_t, st_r = (S1, row0) if row0 < 128 else (S2, row0 - 128)
                  _sgs = int(os.environ.get("KERNEL_SGSPLIT", "0")) or \
                      (ch == NCH - 1 and int(os.environ.get("KERNEL_SGLAST", "1")))
                  for g in range(G):
                      q = nc.gpsimd if (_sgs and g % 2) else nc.sync
                      q.dma_start(
                          st_t[st_r:st_r + 32, g * W:(g + 1) * W],
                          cstg[g:g + 1, :].rearrange("p (h w) -> p h w", w=W))

              from concourse import bass_isa
              rr = epi.tile([64, JD * G], BF16, tag="rr")
              ppool = epi.tile([G, 8], BF16, tag="ppool")
              arr = epi.tile([64, JD * G], BF16, tag="arr")

              def mx_path_s1():
                  sm1 = epi.tile([128, G * W], BF16, tag="sm1")
                  nc.vector.tensor_tensor(sm1[:], S1[:], mnegs[:, 0:G * W], OP.add)
                  r1 = epi.tile([128, G], BF16, tag="r1")
                  nc.vector.tensor_reduce(
                      r1[:], sm1[:].rearrange("p (g w) -> p g w", w=W),
                      AX.X, OP.max)
                  nc.gpsimd.dma_start(rr[:, 0:G], r1[0:64, :])
                  nc.sync.dma_start(rr[:, G:2 * G], r1[64:128, :])
                  nc.gpsimd.partition_all_reduce(
                      arr[:, 0:2 * G], rr[:, 0:2 * G], 64, bass_isa.ReduceOp.max)
                  nc.gpsimd.dma_start(ppool[:, 3:4], arr[0:1, 0:G])
                  nc.sync.dma_start(ppool[:, 4:5], arr[0:1, G:2 * G])

              def mx_path_s2():
                  sm2 = epi.tile([64, G * W], BF16, tag="sm2")
                  nc.vector.tensor_tensor(sm2[:], S2[:],
                                          mnegs[0:64, G * W:2 * G * W], OP.add)
                  r2 = epi.tile([64, G], BF16, tag="r2")
                  nc.vector.tensor_reduce(
                      r2[:], sm2[:].rearrange("p (g w) -> p g w", w=W),
                      AX.X, OP.max)
                  nc.gpsimd.partition_all_reduce(
                      arr[:, 2 * G:3 * G], r2[:], 64, bass_isa.ReduceOp.max)
                  # transpose [1,8] -> [8,1] on PE (outer product with 1.0)
                  # instead of a ~2.5us transposing DMA
                  pmx = pc.tile([8, 1], F32, tag="cp")
                  nc.tensor.matmul(pmx[:], lhsT=arr[0:1, 2 * G:3 * G],
                                   rhs=mlpw[0:1, 16:17], start=True, stop=True)
                  nc.vector.tensor_copy(ppool[:, 7:8], pmx[:])

              gb1 = epi.tile([128, 8], BF16, tag="gb1")
              gb2 = epi.tile([64, 8], BF16, tag="gb2")

              def epi01():
                  # channel attention for j0/j1: their chunks (0-3) are done
                  avgr01 = epi.tile([G, 2], F32, tag="avgr01")
                  nc.vector.tensor_reduce(
                      avgr01[:], acc24[:, 0:16].rearrange("p (j r) -> p j r", r=8),
                      AX.X, OP.add)
                  nc.vector.tensor_tensor(ppool[:, 0:2], avgr01[:],
                                          invc[:, 0:2], OP.mult)
                  z1p01 = pc.tile([4, 4], F32, tag="cp")
                  nc.tensor.matmul(z1p01[:, 0:2], lhsT=mlpw[:, 0:4],
                                   rhs=ppool[:, 0:2], start=True, stop=True)
                  nc.tensor.matmul(z1p01[:, 2:4], lhsT=mlpw[:, 0:4],
                                   rhs=ppool[:, 3:5], start=True, stop=True)
                  nc.scalar.activation(z1ones[0:4, 0:2], z1p01[:, 0:2], AF.Relu,
                                       bias=mlpw[0:4, 12:13], scale=1.0)
                  nc.scalar.activation(z1ones[0:4, 3:5], z1p01[:, 2:4], AF.Relu,
                                       bias=mlpw[0:4, 12:13], scale=1.0)
                  gp01 = pc.tile([2, 8], F32, tag="cp")
                  nc.tensor.matmul(gp01[:], lhsT=z1ones[0:5, 0:2],
                                   rhs=mlpw[0:5, 4:12], start=True, stop=False)
                  nc.tensor.matmul(gp01[:], lhsT=z1ones[0:5, 3:5],
                                   rhs=mlpw[0:5, 4:12], start=False, stop=True)
                  # sigmoid via exp + reciprocal: stays on act table 6
                  eg01 = epi.tile([2, 8], F32, tag="r1")
                  nc.scalar.activation(eg01[:], gp01[:], AF.Exp,
                                       bias=0.0, scale=-1.0)
                  egp01 = epi.tile([2, 8], F32, tag="r2")
                  nc.vector.tensor_scalar_add(egp01[:], eg01[:], 1.0)
                  chg01 = epi.tile([2, 8], BF16, tag="chg01")
                  with nc.allow_low_precision(reason="bf16 channel gate"):
                      nc.vector.reciprocal(chg01[:], egp01[:])
                  for jj, q in ((0, nc.gpsimd), (1, nc.sync)):
                      q.dma_start(
                          gb1[64 * jj:64 * jj + 64, :],
                          chg01[jj:jj + 1, :].unsqueeze(1)
                          .broadcast_to((1, 64, 8)))

              def s_gate(Sg, gb, rows, obase, tg):
                  Sgf = epi.tile([rows, G * W], BF16,
                                 tag="sm1" if rows == 128 else "sm2")
                  gview = gb[0:rows, :].unsqueeze(2).broadcast_to((rows, G, W))
                  nc.vector.tensor_tensor(
                      Sgf[:].rearrange("p (a b) -> p a b", b=W),
                      Sg[:].rearrange("p (a b) -> p a b", b=W), gview, OP.mult)
                  sv = Sgf[:].rearrange("p (g w) -> p w g", g=G)
                  ssum = epi.tile([rows, W], BF16, tag="ss" + tg)
                  smx = epi.tile([rows, W], BF16, tag="sm" + tg)
                  if tg == "b":
                      # pairwise trees at the DVE 2x TT rate instead of 1x
                      # reduces (flat column slices, distinct outputs only)
                      a4 = epi.tile([rows, 4 * W], BF16, tag="sm1")
                      a2 = epi.tile([rows, 2 * W], BF16, tag="scr4k")
                      with nc.allow_low_precision(reason="bf16 channel mean"):
                          nc.vector.tensor_tensor(
                              a4[:], Sgf[:, 0:4 * W], Sgf[:, 4 * W:8 * W],
                              OP.add)
                          nc.vector.tensor_tensor(
                              a2[:], a4[:, 0:2 * W], a4[:, 2 * W:4 * W], OP.add)
                          nc.vector.tensor_tensor(
                              ssum[:], a2[:, 0:W], a2[:, W:2 * W], OP.add)
                      m4 = epi.tile([rows, 4 * W], BF16, tag="sm1")
                      m2 = epi.tile([rows, 2 * W], BF16, tag="scr4k")
                      nc.vector.tensor_tensor(
                          m4[:], Sgf[:, 0:4 * W], Sgf[:, 4 * W:8 * W], OP.max)
                      nc.vector.tensor_tensor(
                          m2[:], m4[:, 0:2 * W], m4[:, 2 * W:4 * W], OP.max)
                      nc.vector.tensor_tensor(
                          smx[:], m2[:, 0:W], m2[:, W:2 * W], OP.max)
                  else:
                      with nc.allow_low_precision(reason="8-term channel mean"):
                          nc.vector.tensor_reduce(ssum[:], sv, AX.X, OP.add)
                      nc.vector.tensor_reduce(smx[:], sv, AX.X, OP.max)
                  q1 = epi.tile([rows, W], BF16, tag="q1" + tg)
                  nc.vector.tensor_scalar_mul(q1[:], smx[:], wsp[0:rows, 1:2])
                  gi = epi.tile([rows, W], BF16, tag="gi" + tg)
                  nc.vector.scalar_tensor_tensor(gi[:], ssum[:], wsp[0:rows, 0:1],
                                                 q1[:], OP.mult, OP.add)
                  # sigmoid(gi + b_sp) = 1/(1 + e^(-gi - b_sp)); wsp col3 = -b_sp
                  ei = epi.tile([rows, W], BF16, tag="sm" + tg)
                  nc.scalar.activation(ei[:], gi[:], AF.Exp,
                                       bias=wsp[0:rows, 3:4], scale=-1.0)
                  e1 = epi.tile([rows, W], BF16, tag="q1" + tg)
                  nc.vector.tensor_scalar_add(e1[:], ei[:], 1.0)
                  sg = epi.tile([rows, W], BF16, tag="gi" + tg)
                  with nc.allow_low_precision(reason="bf16 spatial gate"):
                      nc.vector.reciprocal(sg[:], e1[:])
                  O = epi.tile([rows, G * W], BF16, tag="scr4k")
                  oview = sg[:].unsqueeze(1).broadcast_to((rows, G, W))
                  nc.vector.tensor_tensor(
                      O[:].rearrange("p (a b) -> p a b", b=W),
                      Sgf[:].rearrange("p (a b) -> p a b", b=W), oview, OP.mult)
                  nc.gpsimd.dma_start(
                      out_d[obase:obase + rows].rearrange("r g w -> r (g w)"), O[:])

              # ---- software-pipelined chunk schedule ----
              # emission order: s4(k) | tail(k-1) | pairs01(k) | s4(k+1) |
              #                 pairs23(k) | tail(k) | ...
              prev = None
              pend = None       # (ch, st, htiles, csm) with pairs23 pending
              for ch in range(NCH):
                  st = phase_s4(ch)
                  if pend is not None:
                      phase_pairs(pend[1], range(2, 4), pend[2], pend[3],
                                  _btm=_btlist[pend[0]])
                      prev = (pend[0], pend[1], (pend[2], pend[3]))
                      pend = None
                  if prev is not None:
                      phase_tail(prev[0], prev[1], prev[2])
                      if prev[0] == 3:
                          mx_path_s1()
                          epi01()
                      if prev[0] == 4:
                          s_gate(S1, gb1, 128, 0, "a")
                      prev = None
                  htiles = []
                  csm = phase_pairs(st, range(0, 2), htiles, _btm=_btlist[ch])
                  pend = (ch, st, htiles, csm)
              phase_pairs(pend[1], range(2, 4), pend[2], pend[3],
                          _btm=_btlist[pend[0]])
              phase_tail(pend[0], pend[1], (pend[2], pend[3]))
              mx_path_s2()

              # ---------- j2 epilogue (only piece left after last tail) ----
              avgr2 = epi.tile([G, 1], F32, tag="avgr2")
              nc.vector.tensor_reduce(avgr2[:], acc24[:, 16:24], AX.X, OP.add)
              nc.vector.tensor_tensor(ppool[:, 6:7], avgr2[:], invc[:, 2:3],
                                      OP.mult)
              z1p2 = pc.tile([4, 2], F32, tag="cp")
              nc.tensor.matmul(z1p2[:], lhsT=mlpw[:, 0:4], rhs=ppool[:, 6:8],
                               start=True, stop=True)
              nc.scalar.activation(z1ones[0:4, 2:3], z1p2[:, 0:1], AF.Relu,
                                   bias=mlpw[0:4, 12:13], scale=1.0)
              nc.scalar.activation(z1ones[0:4, 5:6], z1p2[:, 1:2], AF.Relu,
                                   bias=mlpw[0:4, 12:13], scale=1.0)
              gp2 = pc.tile([1, 8], F32, tag="cp")
              nc.tensor.matmul(gp2[:], lhsT=z1ones[0:5, 2:3], rhs=mlpw[0:5, 4:12],
                               start=True, stop=False)
              nc.tensor.matmul(gp2[:], lhsT=z1ones[0:5, 5:6], rhs=mlpw[0:5, 4:12],
                               start=False, stop=True)
              eg2 = epi.tile([1, 8], F32, tag="r1")
              nc.scalar.activation(eg2[:], gp2[:], AF.Exp, bias=0.0, scale=-1.0)
              egp2 = epi.tile([1, 8], F32, tag="avgr01")
              nc.vector.tensor_scalar_add(egp2[:], eg2[:], 1.0)
              chg2 = epi.tile([1, 8], BF16, tag="chg2")
              with nc.allow_low_precision(reason="bf16 channel gate"):
                  nc.vector.reciprocal(chg2[:], egp2[:])
              nc.gpsimd.partition_broadcast(gb2[:], chg2[:], channels=64)
              s_gate(S2, gb2, 64, 128, "b")
            for _it in range(_ITERS):
                _one_iter()

    nc.compile()
    return nc


def _host_inputs(inputs):
    """Build the 8 per-core input maps from the full problem inputs."""
    import ml_dtypes
    L = _f32(inputs["featuresL"])[0]          # [C,H,W]
    R = _f32(inputs["featuresR"])[0]
    W_in = _f32(inputs["W_in"])
    W_dt = _f32(inputs["W_dt"])
    b_dt = _f32(inputs["b_dt"])
    W_B = _f32(inputs["W_B"])
    W_C = _f32(inputs["W_C"])
    A = -np.exp(_f32(inputs["A_log"]))        # [E,S]
    D_skip = _f32(inputs["D_skip"])
    W_out = _f32(inputs["W_out"])
    W1, b1 = _f32(inputs["W1"]), _f32(inputs["b1"])
    W2, b2 = _f32(inputs["W2"]), _f32(inputs["b2"])
    w_sp, b_sp = _f32(inputs["w_sp"]), _f32(inputs["b_sp"])

    # stationary weights [64, 576]
    idx = np.arange(128) % 64
    wse = np.zeros((2 * C, 576), np.float32)
    wse[0:32, 0:128] = W_in[0::2][:, idx]
    wse[32:64, 128:256] = W_in[1::2][:, idx]
    wse[0:32, 256:384] = W_dt[0::2][:, idx]
    wse[32:64, 384:512] = W_dt[1::2][:, idx]
    W_comb = W_in @ (D_skip[:, None] * W_out)        # [64(c), G]
    wse[0:32, 512:520] = W_comb[0::2]
    wse[0:32, 520:528] = W_B[0::2]
    wse[0:32, 528:536] = W_C[0::2]
    wse[32:64, 544:552] = W_comb[1::2]
    wse[32:64, 552:560] = W_B[1::2]
    wse[32:64, 560:568] = W_C[1::2]

    # bf16 stationaries [128, 48]
    wbf = np.zeros((128, 48), np.float32)
    for row in range(128):
        cc, e = divmod(row, 64)
        for q in range(32):
            c2, g = q // 16, q % 16
            if g < 8 and cc == c2:
                wbf[row, q] = W_out[e, g]
    for p4 in range(4):
        for local in range(32):
            c2, g = local // 16, local % 16
            if g < 8:
                wbf[32 * p4 + local, 32 + g] = 1.0
    wbf[0:8, 40:48] = np.eye(8, dtype=np.float32)

    avec = np.zeros((128, 8), np.float32)
    avec[:, 0] = b_dt[idx]
    for p4 in range(4):
        cc = np.arange(128) // 64
        avec[:, 1 + p4] = A[idx, 2 * p4 + cc]

    wspv = np.zeros((128, 4), np.float32)
    wspv[:, 0] = w_sp[0] / G
    wspv[:, 1] = w_sp[1]
    wspv[:, 2] = np.float32(np.asarray(b_sp).reshape(-1)[0]) if np.asarray(b_sp).size else 0.0
    wspv[:, 3] = -wspv[:, 2]

    mlpv = np.zeros((G, 24), np.float32)
    mlpv[:, 0:4] = W1
    mlpv[0:4, 4:12] = W2
    mlpv[4, 4:12] = 2.0 * b2
    mlpv[0:4, 12] = b1
    mlpv[0, 16:19] = 1.0

    maps = []
    wi = np.arange(W)
    for k in range(NCORES):
        d0 = JD * k
        Rsh = np.zeros_like(R)
        if d0 > 0:
            Rsh[:, :, d0:] = R[:, :, :-d0]
        else:
            Rsh = R
        feat = np.zeros((C, 2 * FROW), np.float32)
        feat[:, PAD:PAD + HW] = L.reshape(C, HW)
        feat[:, FROW + PAD:] = Rsh.reshape(C, HW)

        umask = np.zeros((32, JD * W), np.float32)
        for j in range(JD):
            umask[:, j * W:(j + 1) * W] = (wi >= d0 + j).astype(np.float32)[None]

        # S-layout max-pool masks: S1 rows (j=0,1), S2 rows (j=2)
        mnegs = np.full((128, 2 * G * W), -1e30, np.float32)
        for j in range(2):
            row_mask = np.where(wi >= d0 + j, 0.0, -1e30)          # [W]
            mnegs[j * 64:(j + 1) * 64, 0:G * W] = np.tile(row_mask, G)[None]
        mnegs[0:64, G * W:2 * G * W] = np.tile(
            np.where(wi >= d0 + 2, 0.0, -1e30), G)[None]

        invc = np.zeros((G, JD), np.float32)
        for j in range(JD):
            invc[:, j] = 1.0 / (H * (W - (d0 + j)))

        maps.append({
            "feat": feat.astype(ml_dtypes.bfloat16),
            "wse": wse.astype(ml_dtypes.bfloat16),
            "wbf": wbf.astype(ml_dtypes.bfloat16),
            "avec": avec,
            "umask": umask.astype(ml_dtypes.bfloat16),
            "mnegs": mnegs.astype(ml_dtypes.bfloat16),
            "invc": invc,
            "wsp": wspv,
            "mlp": mlpv.astype(ml_dtypes.bfloat16),
        })
    return maps


def kernel(**inputs):
    from concourse.bass_utils import run_bass_kernel_spmd

    if "nc" not in _compiled:
        _compiled["nc"] = _build_program()
    nc = _compiled["nc"]

    maps = _host_inputs(inputs)
    res = run_bass_kernel_spmd(nc, maps, list(range(NCORES))).results

    vol = np.zeros((1, G, DV, H, W), np.float32)
    for k in range(NCORES):
        o = np.asarray(res[k]["out"], np.float32).reshape(JD, H, G, W)        # [j,h,g,w]
        vol[0, :, JD * k:JD * k + JD] = np.transpose(o, (2, 0, 1, 3))
    return vol



# revision 42
# speedup vs baseline: 1.0139x; 1.0119x over previous
"""Trainium2 Bass kernel for nn_BuildCostVolume (stereo cost volume + Mamba scan).

Sharding: disparity axis (24) split as 3 per core across 8 cores; core k
handles disparities d = 3k+j (j in 0..2, compile-time; host pre-shifts
featuresR by 3k so the SPMD program is identical across cores).

Per-core pipeline (software-pipelined across the 6 (j, h-half) chunks as
s4(k) | tail(k-1) | pairs01(k) | s4(k+1) | pairs23(k) | tail(k) | ...):
  - Features loaded once as bf16; u/dt/B/C/D projections on PE from L and
    shifted-R views with even/odd split weights (channel interleave trick).
  - dt = softplus via Exp + Ln(x+1) on ACT; u evicted via ACT Copy so the
    dt*u multiply runs at the DVE 2x (16-bit) rate.
  - Decay a = exp(A*dt) via ACT per-partition scale in an (s-pair x e)
    128-partition layout; B broadcast via SBUF-to-SBUF DMA (Pool queue);
    b = dt*u*B on DVE, with pairs 1,3 offloaded to GPSIMD to balance.
  - Mamba recurrence h = a*h + b via DVE tensor_tensor_scan over flattened
    (row, w) with a[w=0]=0 so each image row restarts the scan.
  - y/cost contraction on PE (block-diag W_out fold, C multiply at PSUM
    eviction, partition-sum + D-term matmul); cstg evicted on ACT with
    fused avg-pool accumulation.
  - Channel-attn max pool from the spatial S layout: masked add + per-g
    max on 128 partitions, GPSIMD partition_all_reduce, tiny transposing
    DMAs; MLP in bf16; spatial attention as in the reference.
  - Output written [j*64+h, g, w] bf16 and transposed/cast on host.
"""
import os
import numpy as np

C, H, W, DV = 32, 64, 128, 24
_NCH_ENV = int(os.environ.get("KERNEL_NCH", "6"))
_SKIP_EPI = bool(int(os.environ.get("KERNEL_SKIP_EPI", "0")))
_SKIP_PAIRS = bool(int(os.environ.get("KERNEL_SKIP_PAIRS", "0")))
_ITERS = int(os.environ.get("KERNEL_ITERS", "1"))
E, S, G = 64, 8, 8
NCORES, JD = 8, 3          # cores, disparities per core
PAD = 8                    # leading zero columns in feature tensors
HH = 32                    # h rows per chunk
NCH = 6                    # chunks = (j, h-half)
CCOLS = HH * W             # 4096 columns per chunk
HW = H * W                 # 8192
FROW = PAD + HW            # 8200 cols per feature image

_compiled = {}


def _f32(x):
    return np.ascontiguousarray(np.asarray(x, np.float32))


def _build_program():
    import concourse.bacc as bacc
    import concourse.mybir as mybir
    from concourse.tile import TileContext

    F32 = mybir.dt.float32
    BF16 = mybir.dt.bfloat16
    AF = mybir.ActivationFunctionType
    AX = mybir.AxisListType
    OP = mybir.AluOpType

    nc = bacc.Bacc("TRN2", target_bir_lowering=False, debug=False,
                   num_devices=NCORES)

    feat_d = nc.dram_tensor("feat", [C, 2 * FROW], BF16, kind="ExternalInput").ap()
    wse_d = nc.dram_tensor("wse", [2 * C, 576], BF16, kind="ExternalInput").ap()
    wbf_d = nc.dram_tensor("wbf", [128, 48], BF16, kind="ExternalInput").ap()
    avec_d = nc.dram_tensor("avec", [128, 8], F32, kind="ExternalInput").ap()
    umask_d = nc.dram_tensor("umask", [32, JD * W], BF16, kind="ExternalInput").ap()
    mnegs_d = nc.dram_tensor("mnegs", [128, 2 * G * W], BF16, kind="ExternalInput").ap()
    invc_d = nc.dram_tensor("invc", [G, JD], F32, kind="ExternalInput").ap()
    wsp_d = nc.dram_tensor("wsp", [128, 4], F32, kind="ExternalInput").ap()
    mlp_d = nc.dram_tensor("mlp", [G, 24], BF16, kind="ExternalInput").ap()
    out_d = nc.dram_tensor("out", [JD * H, G, W], BF16, kind="ExternalOutput").ap()

    with TileContext(nc) as tc:
        with tc.tile_pool(name="const", bufs=1) as cpool, \
             tc.tile_pool(name="dtmp", bufs=1) as dtmpp, \
             tc.tile_pool(name="dt2", bufs=2) as dt2p, \
             tc.tile_pool(name="dtu2", bufs=2) as dtu2p, \
             tc.tile_pool(name="bc", bufs=2) as bcp, \
             tc.tile_pool(name="bb", bufs=2) as bbp, \
             tc.tile_pool(name="csm", bufs=1) as csmp, \
             tc.tile_pool(name="apool", bufs=int(os.environ.get("KERNEL_AB", "2"))) as apl, \
             tc.tile_pool(name="bpool", bufs=int(os.environ.get("KERNEL_BB", "2"))) as bpl, \
             tc.tile_pool(name="hpool", bufs=int(os.environ.get("KERNEL_HB", "4"))) as hpl, \
             tc.tile_pool(name="tpool", bufs=1) as tpl, \
             tc.tile_pool(name="cstg", bufs=1) as cstgp, \
             tc.tile_pool(name="epi", bufs=1) as epi, \
             tc.tile_pool(name="pproj", bufs=2, space="PSUM") as pproj, \
             tc.tile_pool(name="pz", bufs=1, space="PSUM") as pz, \
             tc.tile_pool(name="pc", bufs=1, space="PSUM") as pc:

            _ld = mybir.InstLoadActFuncSet(
                name=nc.get_next_instruction_name(), act_func_set_id=6,
                ins=[], outs=[])
            nc.scalar.add_instruction(_ld)
            wseL = cpool.tile([C, 576], BF16)
            nc.sync.dma_start(wseL[:], wse_d[0:C, :])
            wseR = cpool.tile([C, 576], BF16)
            nc.sync.dma_start(wseR[:], wse_d[C:2 * C, :])
            avec = cpool.tile([128, 8], F32)
            nc.sync.dma_start(avec[:], avec_d[:])
            featsb = cpool.tile([C, 2 * FROW], BF16)
            HB = PAD + HH * W
            nc.sync.dma_start(featsb[:, 0:HB], feat_d[:, 0:HB])
            nc.gpsimd.dma_start(featsb[:, FROW:FROW + HB],
                                feat_d[:, FROW:FROW + HB])
            nc.sync.dma_start(featsb[:, HB:FROW], feat_d[:, HB:FROW])
            nc.gpsimd.dma_start(featsb[:, FROW + HB:2 * FROW],
                                feat_d[:, FROW + HB:2 * FROW])
            wbf = cpool.tile([128, 48], BF16)
            nc.sync.dma_start(wbf[:], wbf_d[:])
            umask = cpool.tile([32, JD * W], BF16)
            nc.sync.dma_start(umask[:], umask_d[:])
            mnegs = cpool.tile([128, 2 * G * W], BF16)
            nc.sync.dma_start(mnegs[:], mnegs_d[:])
            invc = cpool.tile([G, JD], F32)
            nc.sync.dma_start(invc[:], invc_d[:])
            wsp = cpool.tile([128, 4], F32)
            nc.sync.dma_start(wsp[:], wsp_d[:])
            mlpw = cpool.tile([G, 24], BF16)
            nc.sync.dma_start(mlpw[:], mlp_d[:])
            z1ones = cpool.tile([5, 8], BF16)
            nc.sync.dma_start(z1ones[4:5, 0:6], mlpw[0:1, 16:22])

            def _one_iter():
              acc24 = epi.tile([G, 24], F32, tag="acc24")    # per-(chunk,s4) sums
              S1 = epi.tile([128, G * W], BF16, tag="S1")    # spatial rows 0-127
              S2 = epi.tile([64, G * W], BF16, tag="S2")     # spatial rows 128-191
              _btmask = int(os.environ.get("KERNEL_BTPOOL", "10"))
              _btlist = [int(x) for x in os.environ.get(
                  "KERNEL_BTLIST", "10,10,10,10,10,10").split(",")]

              def phase_s4(ch):
                  j, hh = divmod(ch, 2)
                  base = hh * CCOLS
                  dt2 = dt2p.tile([128, CCOLS], BF16)
                  dtu2 = dtu2p.tile([128, CCOLS], BF16)
                  bc = bcp.tile([32, CCOLS], BF16)
                  for s4 in range(4):
                      cs = base + s4 * 1024
                      sl = slice(s4 * 1024, s4 * 1024 + 1024)
                      ftL = featsb[:, PAD + cs: PAD + cs + 1024]
                      ftR = featsb[:, FROW + PAD + cs - j: FROW + PAD + cs - j + 1024]

                      pd = pproj.tile([128, 1024], F32, tag="proj")
                      for hv in range(2):
                          cv = slice(512 * hv, 512 * hv + 512)
                          nc.tensor.matmul(pd[:, cv], lhsT=wseL[:, 256:384],
                                           rhs=ftL[:, cv], start=True, stop=False)
                          nc.tensor.matmul(pd[:, cv], lhsT=wseR[:, 384:512],
                                           rhs=ftR[:, cv], start=False, stop=True)
                      dm = dtmpp.tile([128, 1024], BF16, tag="dm")
                      nc.scalar.activation(dm[:], pd[:], AF.Exp,
                                           bias=avec[:, 0:1], scale=1.0)
                      nc.scalar.activation(dt2[:, sl], dm[:], AF.Ln, bias=1.0,
                                           scale=1.0)

                      pu = pproj.tile([128, 1024], F32, tag="proj")
                      for hv in range(2):
                          cv = slice(512 * hv, 512 * hv + 512)
                          nc.tensor.matmul(pu[:, cv], lhsT=wseL[:, 0:128],
                                           rhs=ftL[:, cv], start=True, stop=False)
                          nc.tensor.matmul(pu[:, cv], lhsT=wseR[:, 128:256],
                                           rhs=ftR[:, cv], start=False, stop=True)
                      if ch == 0 and int(os.environ.get("KERNEL_W0", "1")):
                          nc.vector.tensor_tensor(dtu2[:, sl], dt2[:, sl],
                                                  pu[:], OP.mult)
                      else:
                          u_sb = dtmpp.tile([128, 1024], BF16, tag="usb")
                          nc.scalar.activation(u_sb[:], pu[:], AF.Copy,
                                               bias=0.0, scale=1.0)
                          nc.vector.tensor_tensor(dtu2[:, sl], dt2[:, sl],
                                                  u_sb[:], OP.mult)

                      pb = pproj.tile([128, 1024], F32, tag="proj")
                      for hv in range(2):
                          cv = slice(512 * hv, 512 * hv + 512)
                          nc.tensor.matmul(pb[0:32, cv], lhsT=wseL[:, 512:544],
                                           rhs=ftL[:, cv], start=True, stop=False)
                          nc.tensor.matmul(pb[0:32, cv], lhsT=wseR[:, 544:576],
                                           rhs=ftR[:, cv], start=False, stop=True)
                      mview = umask[:, j * W:(j + 1) * W].unsqueeze(1) \
                          .broadcast_to((32, 8, W))
                      nc.vector.scalar_tensor_tensor(
                          bc[:, sl].rearrange("p (a b) -> p a b", b=W),
                          pb[0:32, :].rearrange("p (a b) -> p a b", b=W), 1.0,
                          mview, OP.mult, OP.mult)
                  return dt2, dtu2, bc

              def phase_pairs(st, prange, htiles, csm=None, _btm=None,
                              last=False):
                  dt2, dtu2, bc = st
                  if _btm is None:
                      _btm = _btmask
                  _bbq = nc.sync if int(os.environ.get("KERNEL_BBSP", "0")) else nc.gpsimd
                  if csm is None:
                      csm = csmp.tile([128, CCOLS], BF16)
                      _bbq.dma_start(
                          csm[:],
                          bc[16:24, :].unsqueeze(1).broadcast_to((8, 16, CCOLS)))
                  for p in prange:
                      bb = bbp.tile([128, CCOLS], BF16)
                      bt = bpl.tile([128, CCOLS], BF16)
                      _sliced = (ch == 0 and
                                 p < int(os.environ.get("KERNEL_WSLICE", "2"))
                                 ) or last
                      if _sliced:
                          for s4 in range(4):
                              sl = slice(s4 * 1024, s4 * 1024 + 1024)
                              _bbq.dma_start(
                                  bb[:, sl],
                                  bc[8 + 2 * p:8 + 2 * p + 2, sl].unsqueeze(1)
                                  .broadcast_to((2, 64, 1024)))
                      else:
                          _bbq.dma_start(
                              bb[:],
                              bc[8 + 2 * p:8 + 2 * p + 2, :].unsqueeze(1)
                              .broadcast_to((2, 64, CCOLS)))
                      av = apl.tile([128, CCOLS], BF16)
                      if last:
                          # per-s4 slices: 1024-col boundaries are row starts
                          # (av[w=0]=0 restarts the scan), so split ops are
                          # exact and range-tracked deps let the tail start
                          # after the first slice
                          for s4 in range(4):
                              sl = slice(s4 * 1024, s4 * 1024 + 1024)
                              nc.scalar.activation(
                                  av[:, sl], dt2[:, sl], AF.Exp,
                                  bias=0.0, scale=avec[:, 1 + p: 2 + p])
                              nc.vector.memset(
                                  av[:, sl].rearrange("p (h w) -> p h w", w=W)
                                  [:, :, 0:1], 0)
                      else:
                          nc.scalar.activation(av[:], dt2[:], AF.Exp,
                                               bias=0.0,
                                               scale=avec[:, 1 + p: 2 + p])
                          nc.vector.memset(
                              av[:].rearrange("p (h w) -> p h w", w=W)
                              [:, :, 0:1], 0)
                      bteng = nc.gpsimd if (_btm >> p) & 1 else nc.vector
                      if _sliced:
                          for s4 in range(4):
                              sl = slice(s4 * 1024, s4 * 1024 + 1024)
                              bteng.tensor_tensor(bt[:, sl], dtu2[:, sl],
                                                  bb[:, sl], OP.mult)
                      else:
                          bteng.tensor_tensor(bt[:], dtu2[:], bb[:], OP.mult)
                      hT = hpl.tile([128, CCOLS], BF16)
                      if last:
                          for s4 in range(4):
                              sl = slice(s4 * 1024, s4 * 1024 + 1024)
                              nc.vector.tensor_tensor_scan(
                                  hT[:, sl], av[:, sl], bt[:, sl], 0.0,
                                  OP.mult, OP.add)
                      else:
                          nc.vector.tensor_tensor_scan(hT[:], av[:], bt[:], 0.0,
                                                       OP.mult, OP.add)
                      htiles.append(hT)
                  return csm

              def phase_tail(ch, st, pr):
                  j, hh = divmod(ch, 2)
                  dt2, dtu2, bc = st
                  htiles, csm = pr
                  tt = tpl.tile([128, CCOLS], BF16, tag="tt")
                  for s8 in range(4):
                      sl10 = slice(s8 * 1024, s8 * 1024 + 1024)
                      zp = pz.tile([128, 1024], F32, tag="zp")
                      for half in range(2):
                          zv = slice(512 * half, 512 * half + 512)
                          sl5 = slice(s8 * 1024 + 512 * half,
                                      s8 * 1024 + 512 * half + 512)
                          for p in range(4):
                              nc.tensor.matmul(zp[32 * p:32 * p + 32, zv],
                                               lhsT=wbf[:, 0:32],
                                               rhs=htiles[p][:, sl5],
                                               start=True, stop=True,
                                               tile_position=(0, 32 * p))
                      _tta = int(os.environ.get("KERNEL_TTACT", "4"))
                      if ch == NCH - 1 and int(os.environ.get("KERNEL_T5", "1")):
                          _tta = 0
                      if s8 < _tta:
                          z_sb = epi.tile([128, 1024], BF16, tag="scr4k")
                          nc.scalar.activation(z_sb[:], zp[:], AF.Copy,
                                               bias=0.0, scale=1.0)
                          nc.vector.tensor_tensor(tt[:, sl10], z_sb[:],
                                                  csm[:, sl10], OP.mult)
                      else:
                          nc.vector.scalar_tensor_tensor(tt[:, sl10], zp[:], 1.0,
                                                         csm[:, sl10], OP.mult, OP.mult)

                  if int(os.environ.get("KERNEL_CSHARE", "0")):
                      cstg = tpl.tile([8, CCOLS], BF16, tag="tt")
                  else:
                      cstg = cstgp.tile([8, CCOLS], BF16)
                  for s4 in range(4):
                      sl = slice(s4 * 1024, s4 * 1024 + 1024)
                      cp = pc.tile([8, 1024], F32, tag="cp")
                      for hv in range(2):
                          cv = slice(512 * hv, 512 * hv + 512)
                          cg = slice(s4 * 1024 + 512 * hv, s4 * 1024 + 512 * hv + 512)
                          nc.tensor.matmul(cp[:, cv], lhsT=wbf[:, 32:40],
                                           rhs=tt[:, cg], start=True, stop=False)
                          nc.tensor.matmul(cp[:, cv], lhsT=wbf[0:8, 40:48],
                                           rhs=bc[0:8, cg], start=False, stop=True)
                      nc.scalar.activation(
                          cstg[:, sl], cp[:], AF.Copy, bias=0.0, scale=1.0,
                          accum_out=acc24[:, ch * 4 + s4: ch * 4 + s4 + 1])

                  row0 = j * 64 + hh * 32
                  st_t, st_r = (S1, row0) if row0 < 128 else (S2, row0 - 128)
                  _sgs = int(os.environ.get("KERNEL_SGSPLIT", "0")) or \
                      (ch == NCH - 1 and int(os.environ.get("KERNEL_SGLAST", "1")))
                  for g in range(G):
                      q = nc.gpsimd if (_sgs and g % 2) else nc.sync
                      q.dma_start(
                          st_t[st_r:st_r + 32, g * W:(g + 1) * W],
                          cstg[g:g + 1, :].rearrange("p (h w) -> p h w", w=W))

              from concourse import bass_isa
              rr = epi.tile([64, JD * G], BF16, tag="rr")
              ppool = epi.tile([G, 8], BF16, tag="ppool")
              arr = epi.tile([64, JD * G], BF16, tag="arr")

              def mx_path_s1():
                  sm1 = epi.tile([128, G * W], BF16, tag="sm1")
                  nc.vector.tensor_tensor(sm1[:], S1[:], mnegs[:, 0:G * W], OP.add)
                  r1 = epi.tile([128, G], BF16, tag="r1")
                  nc.vector.tensor_reduce(
                      r1[:], sm1[:].rearrange("p (g w) -> p g w", w=W),
                      AX.X, OP.max)
                  nc.gpsimd.dma_start(rr[:, 0:G], r1[0:64, :])
                  nc.sync.dma_start(rr[:, G:2 * G], r1[64:128, :])
                  nc.gpsimd.partition_all_reduce(
                      arr[:, 0:2 * G], rr[:, 0:2 * G], 64, bass_isa.ReduceOp.max)
                  nc.gpsimd.dma_start(ppool[:, 3:4], arr[0:1, 0:G])
                  nc.sync.dma_start(ppool[:, 4:5], arr[0:1, G:2 * G])

              def mx_path_s2():
                  sm2 = epi.tile([64, G * W], BF16, tag="sm2")
                  nc.vector.tensor_tensor(sm2[:], S2[:],
                                          mnegs[0:64, G * W:2 * G * W], OP.add)
                  r2 = epi.tile([64, G], BF16, tag="r2")
                  nc.vector.tensor_reduce(
                      r2[:], sm2[:].rearrange("p (g w) -> p g w", w=W),
                      AX.X, OP.max)
                  nc.gpsimd.partition_all_reduce(
                      arr[:, 2 * G:3 * G], r2[:], 64, bass_isa.ReduceOp.max)
                  # transpose [1,8] -> [8,1] on PE (outer product with 1.0)
                  # instead of a ~2.5us transposing DMA
                  pmx = pc.tile([8, 1], F32, tag="cp")
                  nc.tensor.matmul(pmx[:], lhsT=arr[0:1, 2 * G:3 * G],
                                   rhs=mlpw[0:1, 16:17], start=True, stop=True)
                  nc.vector.tensor_copy(ppool[:, 7:8], pmx[:])

              gb1 = epi.tile([128, 8], BF16, tag="gb1")
              gb2 = epi.tile([64, 8], BF16, tag="gb2")

              def epi01():
                  # channel attention for j0/j1: their chunks (0-3) are done
                  avgr01 = epi.tile([G, 2], F32, tag="avgr01")
                  nc.vector.tensor_reduce(
                      avgr01[:], acc24[:, 0:16].rearrange("p (j r) -> p j r", r=8),
                      AX.X, OP.add)
                  nc.vector.tensor_tensor(ppool[:, 0:2], avgr01[:],
                                          invc[:, 0:2], OP.mult)
                  z1p01 = pc.tile([4, 4], F32, tag="cp")
                  nc.tensor.matmul(z1p01[:, 0:2], lhsT=mlpw[:, 0:4],
                                   rhs=ppool[:, 0:2], start=True, stop=True)
                  nc.tensor.matmul(z1p01[:, 2:4], lhsT=mlpw[:, 0:4],
                                   rhs=ppool[:, 3:5], start=True, stop=True)
                  nc.scalar.activation(z1ones[0:4, 0:2], z1p01[:, 0:2], AF.Relu,
                                       bias=mlpw[0:4, 12:13], scale=1.0)
                  nc.scalar.activation(z1ones[0:4, 3:5], z1p01[:, 2:4], AF.Relu,
                                       bias=mlpw[0:4, 12:13], scale=1.0)
                  gp01 = pc.tile([2, 8], F32, tag="cp")
                  nc.tensor.matmul(gp01[:], lhsT=z1ones[0:5, 0:2],
                                   rhs=mlpw[0:5, 4:12], start=True, stop=False)
                  nc.tensor.matmul(gp01[:], lhsT=z1ones[0:5, 3:5],
                                   rhs=mlpw[0:5, 4:12], start=False, stop=True)
                  # sigmoid via exp + reciprocal: stays on act table 6
                  eg01 = epi.tile([2, 8], F32, tag="r1")
                  nc.scalar.activation(eg01[:], gp01[:], AF.Exp,
                                       bias=0.0, scale=-1.0)
                  egp01 = epi.tile([2, 8], F32, tag="r2")
                  nc.vector.tensor_scalar_add(egp01[:], eg01[:], 1.0)
                  chg01 = epi.tile([2, 8], BF16, tag="chg01")
                  with nc.allow_low_precision(reason="bf16 channel gate"):
                      nc.vector.reciprocal(chg01[:], egp01[:])
                  for jj, q in ((0, nc.gpsimd), (1, nc.sync)):
                      q.dma_start(
                          gb1[64 * jj:64 * jj + 64, :],
                          chg01[jj:jj + 1, :].unsqueeze(1)
                          .broadcast_to((1, 64, 8)))

              def s_gate(Sg, gb, rows, obase, tg):
                  Sgf = epi.tile([rows, G * W], BF16,
                                 tag="sm1" if rows == 128 else "sm2")
                  gview = gb[0:rows, :].unsqueeze(2).broadcast_to((rows, G, W))
                  nc.vector.tensor_tensor(
                      Sgf[:].rearrange("p (a b) -> p a b", b=W),
                      Sg[:].rearrange("p (a b) -> p a b", b=W), gview, OP.mult)
                  sv = Sgf[:].rearrange("p (g w) -> p w g", g=G)
                  ssum = epi.tile([rows, W], BF16, tag="ss" + tg)
                  smx = epi.tile([rows, W], BF16, tag="sm" + tg)
                  if tg == "b":
                      # pairwise trees at the DVE 2x TT rate instead of 1x
                      # reduces (flat column slices, distinct outputs only)
                      a4 = epi.tile([rows, 4 * W], BF16, tag="sm1")
                      a2 = epi.tile([rows, 2 * W], BF16, tag="scr4k")
                      with nc.allow_low_precision(reason="bf16 channel mean"):
                          nc.vector.tensor_tensor(
                              a4[:], Sgf[:, 0:4 * W], Sgf[:, 4 * W:8 * W],
                              OP.add)
                          nc.vector.tensor_tensor(
                              a2[:], a4[:, 0:2 * W], a4[:, 2 * W:4 * W], OP.add)
                          nc.vector.tensor_tensor(
                              ssum[:], a2[:, 0:W], a2[:, W:2 * W], OP.add)
                      m4 = epi.tile([rows, 4 * W], BF16, tag="sm1")
                      m2 = epi.tile([rows, 2 * W], BF16, tag="scr4k")
                      nc.vector.tensor_tensor(
                          m4[:], Sgf[:, 0:4 * W], Sgf[:, 4 * W:8 * W], OP.max)
                      nc.vector.tensor_tensor(
                          m2[:], m4[:, 0:2 * W], m4[:, 2 * W:4 * W], OP.max)
                      nc.vector.tensor_tensor(
                          smx[:], m2[:, 0:W], m2[:, W:2 * W], OP.max)
                  else:
                      with nc.allow_low_precision(reason="8-term channel mean"):
                          nc.vector.tensor_reduce(ssum[:], sv, AX.X, OP.add)
                      nc.vector.tensor_reduce(smx[:], sv, AX.X, OP.max)
                  q1 = epi.tile([rows, W], BF16, tag="q1" + tg)
                  nc.vector.tensor_scalar_mul(q1[:], smx[:], wsp[0:rows, 1:2])
                  gi = epi.tile([rows, W], BF16, tag="gi" + tg)
                  nc.vector.scalar_tensor_tensor(gi[:], ssum[:], wsp[0:rows, 0:1],
                                                 q1[:], OP.mult, OP.add)
                  # sigmoid(gi + b_sp) = 1/(1 + e^(-gi - b_sp)); wsp col3 = -b_sp
                  ei = epi.tile([rows, W], BF16, tag="sm" + tg)
                  nc.scalar.activation(ei[:], gi[:], AF.Exp,
                                       bias=wsp[0:rows, 3:4], scale=-1.0)
                  e1 = epi.tile([rows, W], BF16, tag="q1" + tg)
                  nc.vector.tensor_scalar_add(e1[:], ei[:], 1.0)
                  sg = epi.tile([rows, W], BF16, tag="gi" + tg)
                  with nc.allow_low_precision(reason="bf16 spatial gate"):
                      nc.vector.reciprocal(sg[:], e1[:])
                  O = epi.tile([rows, G * W], BF16, tag="scr4k")
                  oview = sg[:].unsqueeze(1).broadcast_to((rows, G, W))
                  nc.vector.tensor_tensor(
                      O[:].rearrange("p (a b) -> p a b", b=W),
                      Sgf[:].rearrange("p (a b) -> p a b", b=W), oview, OP.mult)
                  nc.gpsimd.dma_start(
                      out_d[obase:obase + rows].rearrange("r g w -> r (g w)"), O[:])

              # ---- software-pipelined chunk schedule ----
              # emission order: s4(k) | tail(k-1) | pairs01(k) | s4(k+1) |
              #                 pairs23(k) | tail(k) | ...
              prev = None
              pend = None       # (ch, st, htiles, csm) with pairs23 pending
              for ch in range(NCH):
                  st = phase_s4(ch)
                  if pend is not None:
                      phase_pairs(pend[1], range(2, 4), pend[2], pend[3],
                                  _btm=_btlist[pend[0]])
                      prev = (pend[0], pend[1], (pend[2], pend[3]))
                      pend = None
                  if prev is not None:
                      phase_tail(prev[0], prev[1], prev[2])
                      if prev[0] == 3:
                          mx_path_s1()
                          epi01()
                      if prev[0] == 4:
                          s_gate(S1, gb1, 128, 0, "a")
                      prev = None
                  htiles = []
                  csm = phase_pairs(st, range(0, 2), htiles, _btm=_btlist[ch])
                  pend = (ch, st, htiles, csm)
              phase_pairs(pend[1], range(2, 4), pend[2], pend[3],
                          _btm=_btlist[pend[0]], last=True)
              phase_tail(pend[0], pend[1], (pend[2], pend[3]))
              mx_path_s2()

              # ---------- j2 epilogue (only piece left after last tail) ----
              avgr2 = epi.tile([G, 1], F32, tag="avgr2")
              nc.vector.tensor_reduce(avgr2[:], acc24[:, 16:24], AX.X, OP.add)
              nc.vector.tensor_tensor(ppool[:, 6:7], avgr2[:], invc[:, 2:3],
                                      OP.mult)
              z1p2 = pc.tile([4, 2], F32, tag="cp")
              nc.tensor.matmul(z1p2[:], lhsT=mlpw[:, 0:4], rhs=ppool[:, 6:8],
                               start=True, stop=True)
              nc.scalar.activation(z1ones[0:4, 2:3], z1p2[:, 0:1], AF.Relu,
                                   bias=mlpw[0:4, 12:13], scale=1.0)
              nc.scalar.activation(z1ones[0:4, 5:6], z1p2[:, 1:2], AF.Relu,
                                   bias=mlpw[0:4, 12:13], scale=1.0)
              gp2 = pc.tile([1, 8], F32, tag="cp")
              nc.tensor.matmul(gp2[:], lhsT=z1ones[0:5, 2:3], rhs=mlpw[0:5, 4:12],
                               start=True, stop=False)
              nc.tensor.matmul(gp2[:], lhsT=z1ones[0:5, 5:6], rhs=mlpw[0:5, 4:12],
                               start=False, stop=True)
              eg2 = epi.tile([1, 8], F32, tag="r1")
              nc.scalar.activation(eg2[:], gp2[:], AF.Exp, bias=0.0, scale=-1.0)
              egp2 = epi.tile([1, 8], F32, tag="avgr01")
              nc.vector.tensor_scalar_add(egp2[:], eg2[:], 1.0)
              chg2 = epi.tile([1, 8], BF16, tag="chg2")
              with nc.allow_low_precision(reason="bf16 channel gate"):
                  nc.vector.reciprocal(chg2[:], egp2[:])
              nc.gpsimd.partition_broadcast(gb2[:], chg2[:], channels=64)
              s_gate(S2, gb2, 64, 128, "b")
            for _it in range(_ITERS):
                _one_iter()

    nc.compile()
    return nc


def _host_inputs(inputs):
    """Build the 8 per-core input maps from the full problem inputs."""
    import ml_dtypes
    L = _f32(inputs["featuresL"])[0]          # [C,H,W]
    R = _f32(inputs["featuresR"])[0]
    W_in = _f32(inputs["W_in"])
    W_dt = _f32(inputs["W_dt"])
    b_dt = _f32(inputs["b_dt"])
    W_B = _f32(inputs["W_B"])
    W_C = _f32(inputs["W_C"])
    A = -np.exp(_f32(inputs["A_log"]))        # [E,S]
    D_skip = _f32(inputs["D_skip"])
    W_out = _f32(inputs["W_out"])
    W1, b1 = _f32(inputs["W1"]), _f32(inputs["b1"])
    W2, b2 = _f32(inputs["W2"]), _f32(inputs["b2"])
    w_sp, b_sp = _f32(inputs["w_sp"]), _f32(inputs["b_sp"])

    # stationary weights [64, 576]
    idx = np.arange(128) % 64
    wse = np.zeros((2 * C, 576), np.float32)
    wse[0:32, 0:128] = W_in[0::2][:, idx]
    wse[32:64, 128:256] = W_in[1::2][:, idx]
    wse[0:32, 256:384] = W_dt[0::2][:, idx]
    wse[32:64, 384:512] = W_dt[1::2][:, idx]
    W_comb = W_in @ (D_skip[:, None] * W_out)        # [64(c), G]
    wse[0:32, 512:520] = W_comb[0::2]
    wse[0:32, 520:528] = W_B[0::2]
    wse[0:32, 528:536] = W_C[0::2]
    wse[32:64, 544:552] = W_comb[1::2]
    wse[32:64, 552:560] = W_B[1::2]
    wse[32:64, 560:568] = W_C[1::2]

    # bf16 stationaries [128, 48]
    wbf = np.zeros((128, 48), np.float32)
    for row in range(128):
        cc, e = divmod(row, 64)
        for q in range(32):
            c2, g = q // 16, q % 16
            if g < 8 and cc == c2:
                wbf[row, q] = W_out[e, g]
    for p4 in range(4):
        for local in range(32):
            c2, g = local // 16, local % 16
            if g < 8:
                wbf[32 * p4 + local, 32 + g] = 1.0
    wbf[0:8, 40:48] = np.eye(8, dtype=np.float32)

    avec = np.zeros((128, 8), np.float32)
    avec[:, 0] = b_dt[idx]
    for p4 in range(4):
        cc = np.arange(128) // 64
        avec[:, 1 + p4] = A[idx, 2 * p4 + cc]

    wspv = np.zeros((128, 4), np.float32)
    wspv[:, 0] = w_sp[0] / G
    wspv[:, 1] = w_sp[1]
    wspv[:, 2] = np.float32(np.asarray(b_sp).reshape(-1)[0]) if np.asarray(b_sp).size else 0.0
    wspv[:, 3] = -wspv[:, 2]

    mlpv = np.zeros((G, 24), np.float32)
    mlpv[:, 0:4] = W1
    mlpv[0:4, 4:12] = W2
    mlpv[4, 4:12] = 2.0 * b2
    mlpv[0:4, 12] = b1
    mlpv[0, 16:19] = 1.0

    maps = []
    wi = np.arange(W)
    for k in range(NCORES):
        d0 = JD * k
        Rsh = np.zeros_like(R)
        if d0 > 0:
            Rsh[:, :, d0:] = R[:, :, :-d0]
        else:
            Rsh = R
        feat = np.zeros((C, 2 * FROW), np.float32)
        feat[:, PAD:PAD + HW] = L.reshape(C, HW)
        feat[:, FROW + PAD:] = Rsh.reshape(C, HW)

        umask = np.zeros((32, JD * W), np.float32)
        for j in range(JD):
            umask[:, j * W:(j + 1) * W] = (wi >= d0 + j).astype(np.float32)[None]

        # S-layout max-pool masks: S1 rows (j=0,1), S2 rows (j=2)
        mnegs = np.full((128, 2 * G * W), -1e30, np.float32)
        for j in range(2):
            row_mask = np.where(wi >= d0 + j, 0.0, -1e30)          # [W]
            mnegs[j * 64:(j + 1) * 64, 0:G * W] = np.tile(row_mask, G)[None]
        mnegs[0:64, G * W:2 * G * W] = np.tile(
            np.where(wi >= d0 + 2, 0.0, -1e30), G)[None]

        invc = np.zeros((G, JD), np.float32)
        for j in range(JD):
            invc[:, j] = 1.0 / (H * (W - (d0 + j)))

        maps.append({
            "feat": feat.astype(ml_dtypes.bfloat16),
            "wse": wse.astype(ml_dtypes.bfloat16),
            "wbf": wbf.astype(ml_dtypes.bfloat16),
            "avec": avec,
            "umask": umask.astype(ml_dtypes.bfloat16),
            "mnegs": mnegs.astype(ml_dtypes.bfloat16),
            "invc": invc,
            "wsp": wspv,
            "mlp": mlpv.astype(ml_dtypes.bfloat16),
        })
    return maps


def kernel(**inputs):
    from concourse.bass_utils import run_bass_kernel_spmd

    if "nc" not in _compiled:
        _compiled["nc"] = _build_program()
    nc = _compiled["nc"]

    maps = _host_inputs(inputs)
    res = run_bass_kernel_spmd(nc, maps, list(range(NCORES))).results

    vol = np.zeros((1, G, DV, H, W), np.float32)
    for k in range(NCORES):
        o = np.asarray(res[k]["out"], np.float32).reshape(JD, H, G, W)        # [j,h,g,w]
        vol[0, :, JD * k:JD * k + JD] = np.transpose(o, (2, 0, 1, 3))
    return vol



# revision 44
# speedup vs baseline: 1.0222x; 1.0082x over previous
"""Trainium2 Bass kernel for nn_BuildCostVolume (stereo cost volume + Mamba scan).

Sharding: disparity axis (24) split as 3 per core across 8 cores; core k
handles disparities d = 3k+j (j in 0..2, compile-time; host pre-shifts
featuresR by 3k so the SPMD program is identical across cores).

Per-core pipeline (software-pipelined across the 6 (j, h-half) chunks as
s4(k) | tail(k-1) | pairs01(k) | s4(k+1) | pairs23(k) | tail(k) | ...):
  - Features loaded once as bf16; u/dt/B/C/D projections on PE from L and
    shifted-R views with even/odd split weights (channel interleave trick).
  - dt = softplus via Exp + Ln(x+1) on ACT; u evicted via ACT Copy so the
    dt*u multiply runs at the DVE 2x (16-bit) rate.
  - Decay a = exp(A*dt) via ACT per-partition scale in an (s-pair x e)
    128-partition layout; B broadcast via SBUF-to-SBUF DMA (Pool queue);
    b = dt*u*B on DVE, with pairs 1,3 offloaded to GPSIMD to balance.
  - Mamba recurrence h = a*h + b via DVE tensor_tensor_scan over flattened
    (row, w) with a[w=0]=0 so each image row restarts the scan.
  - y/cost contraction on PE (block-diag W_out fold, C multiply at PSUM
    eviction, partition-sum + D-term matmul); cstg evicted on ACT with
    fused avg-pool accumulation.
  - Channel-attn max pool from the spatial S layout: masked add + per-g
    max on 128 partitions, GPSIMD partition_all_reduce, tiny transposing
    DMAs; MLP in bf16; spatial attention as in the reference.
  - Output written [j*64+h, g, w] bf16 and transposed/cast on host.
"""
import os
import numpy as np

C, H, W, DV = 32, 64, 128, 24
_NCH_ENV = int(os.environ.get("KERNEL_NCH", "6"))
_SKIP_EPI = bool(int(os.environ.get("KERNEL_SKIP_EPI", "0")))
_SKIP_PAIRS = bool(int(os.environ.get("KERNEL_SKIP_PAIRS", "0")))
_ITERS = int(os.environ.get("KERNEL_ITERS", "1"))
_SL01 = bool(int(os.environ.get("KERNEL_SL01", "1")))
_SLALL = bool(int(os.environ.get("KERNEL_SLALL", "0")))
E, S, G = 64, 8, 8
NCORES, JD = 8, 3          # cores, disparities per core
PAD = 8                    # leading zero columns in feature tensors
HH = 32                    # h rows per chunk
NCH = 6                    # chunks = (j, h-half)
CCOLS = HH * W             # 4096 columns per chunk
HW = H * W                 # 8192
FROW = PAD + HW            # 8200 cols per feature image

_compiled = {}


def _f32(x):
    return np.ascontiguousarray(np.asarray(x, np.float32))


def _build_program():
    import concourse.bacc as bacc
    import concourse.mybir as mybir
    from concourse.tile import TileContext

    F32 = mybir.dt.float32
    BF16 = mybir.dt.bfloat16
    AF = mybir.ActivationFunctionType
    AX = mybir.AxisListType
    OP = mybir.AluOpType

    nc = bacc.Bacc("TRN2", target_bir_lowering=False, debug=False,
                   num_devices=NCORES)

    feat_d = nc.dram_tensor("feat", [C, 2 * FROW], BF16, kind="ExternalInput").ap()
    wse_d = nc.dram_tensor("wse", [2 * C, 576], BF16, kind="ExternalInput").ap()
    wbf_d = nc.dram_tensor("wbf", [128, 48], BF16, kind="ExternalInput").ap()
    avec_d = nc.dram_tensor("avec", [128, 8], F32, kind="ExternalInput").ap()
    umask_d = nc.dram_tensor("umask", [32, JD * W], BF16, kind="ExternalInput").ap()
    mnegs_d = nc.dram_tensor("mnegs", [128, 2 * G * W], BF16, kind="ExternalInput").ap()
    invc_d = nc.dram_tensor("invc", [G, JD], F32, kind="ExternalInput").ap()
    wsp_d = nc.dram_tensor("wsp", [128, 4], F32, kind="ExternalInput").ap()
    mlp_d = nc.dram_tensor("mlp", [G, 24], BF16, kind="ExternalInput").ap()
    out_d = nc.dram_tensor("out", [JD * H, G, W], BF16, kind="ExternalOutput").ap()

    with TileContext(nc) as tc:
        with tc.tile_pool(name="const", bufs=1) as cpool, \
             tc.tile_pool(name="dtmp", bufs=1) as dtmpp, \
             tc.tile_pool(name="dt2", bufs=2) as dt2p, \
             tc.tile_pool(name="dtu2", bufs=2) as dtu2p, \
             tc.tile_pool(name="bc", bufs=2) as bcp, \
             tc.tile_pool(name="bb", bufs=2) as bbp, \
             tc.tile_pool(name="csm", bufs=1) as csmp, \
             tc.tile_pool(name="apool", bufs=int(os.environ.get("KERNEL_AB", "2"))) as apl, \
             tc.tile_pool(name="bpool", bufs=int(os.environ.get("KERNEL_BB", "2"))) as bpl, \
             tc.tile_pool(name="hpool", bufs=int(os.environ.get("KERNEL_HB", "4"))) as hpl, \
             tc.tile_pool(name="tpool", bufs=1) as tpl, \
             tc.tile_pool(name="cstg", bufs=1) as cstgp, \
             tc.tile_pool(name="epi", bufs=1) as epi, \
             tc.tile_pool(name="pproj", bufs=2, space="PSUM") as pproj, \
             tc.tile_pool(name="pz", bufs=1, space="PSUM") as pz, \
             tc.tile_pool(name="pc", bufs=1, space="PSUM") as pc:

            _ld = mybir.InstLoadActFuncSet(
                name=nc.get_next_instruction_name(), act_func_set_id=6,
                ins=[], outs=[])
            nc.scalar.add_instruction(_ld)
            wseL = cpool.tile([C, 576], BF16)
            nc.sync.dma_start(wseL[:], wse_d[0:C, :])
            wseR = cpool.tile([C, 576], BF16)
            nc.sync.dma_start(wseR[:], wse_d[C:2 * C, :])
            avec = cpool.tile([128, 8], F32)
            nc.sync.dma_start(avec[:], avec_d[:])
            featsb = cpool.tile([C, 2 * FROW], BF16)
            HB = PAD + HH * W
            nc.sync.dma_start(featsb[:, 0:HB], feat_d[:, 0:HB])
            nc.gpsimd.dma_start(featsb[:, FROW:FROW + HB],
                                feat_d[:, FROW:FROW + HB])
            nc.sync.dma_start(featsb[:, HB:FROW], feat_d[:, HB:FROW])
            nc.gpsimd.dma_start(featsb[:, FROW + HB:2 * FROW],
                                feat_d[:, FROW + HB:2 * FROW])
            wbf = cpool.tile([128, 48], BF16)
            nc.sync.dma_start(wbf[:], wbf_d[:])
            umask = cpool.tile([32, JD * W], BF16)
            nc.sync.dma_start(umask[:], umask_d[:])
            mnegs = cpool.tile([128, 2 * G * W], BF16)
            nc.sync.dma_start(mnegs[:], mnegs_d[:])
            invc = cpool.tile([G, JD], F32)
            nc.sync.dma_start(invc[:], invc_d[:])
            wsp = cpool.tile([128, 4], F32)
            nc.sync.dma_start(wsp[:], wsp_d[:])
            mlpw = cpool.tile([G, 24], BF16)
            nc.sync.dma_start(mlpw[:], mlp_d[:])
            z1ones = cpool.tile([5, 8], BF16)
            nc.sync.dma_start(z1ones[4:5, 0:6], mlpw[0:1, 16:22])

            def _one_iter():
              acc24 = epi.tile([G, 24], F32, tag="acc24")    # per-(chunk,s4) sums
              S1 = epi.tile([128, G * W], BF16, tag="S1")    # spatial rows 0-127
              S2 = epi.tile([64, G * W], BF16, tag="S2")     # spatial rows 128-191
              _btmask = int(os.environ.get("KERNEL_BTPOOL", "10"))
              _btlist = [int(x) for x in os.environ.get(
                  "KERNEL_BTLIST", "10,10,10,10,10,10").split(",")]

              def phase_s4(ch):
                  j, hh = divmod(ch, 2)
                  base = hh * CCOLS
                  dt2 = dt2p.tile([128, CCOLS], BF16)
                  dtu2 = dtu2p.tile([128, CCOLS], BF16)
                  bc = bcp.tile([32, CCOLS], BF16)
                  for s4 in range(4):
                      cs = base + s4 * 1024
                      sl = slice(s4 * 1024, s4 * 1024 + 1024)
                      ftL = featsb[:, PAD + cs: PAD + cs + 1024]
                      ftR = featsb[:, FROW + PAD + cs - j: FROW + PAD + cs - j + 1024]

                      pd = pproj.tile([128, 1024], F32, tag="proj")
                      for hv in range(2):
                          cv = slice(512 * hv, 512 * hv + 512)
                          nc.tensor.matmul(pd[:, cv], lhsT=wseL[:, 256:384],
                                           rhs=ftL[:, cv], start=True, stop=False)
                          nc.tensor.matmul(pd[:, cv], lhsT=wseR[:, 384:512],
                                           rhs=ftR[:, cv], start=False, stop=True)
                      dm = dtmpp.tile([128, 1024], BF16, tag="dm")
                      nc.scalar.activation(dm[:], pd[:], AF.Exp,
                                           bias=avec[:, 0:1], scale=1.0)
                      nc.scalar.activation(dt2[:, sl], dm[:], AF.Ln, bias=1.0,
                                           scale=1.0)

                      pu = pproj.tile([128, 1024], F32, tag="proj")
                      for hv in range(2):
                          cv = slice(512 * hv, 512 * hv + 512)
                          nc.tensor.matmul(pu[:, cv], lhsT=wseL[:, 0:128],
                                           rhs=ftL[:, cv], start=True, stop=False)
                          nc.tensor.matmul(pu[:, cv], lhsT=wseR[:, 128:256],
                                           rhs=ftR[:, cv], start=False, stop=True)
                      if ch == 0 and int(os.environ.get("KERNEL_W0", "1")):
                          nc.vector.tensor_tensor(dtu2[:, sl], dt2[:, sl],
                                                  pu[:], OP.mult)
                      else:
                          u_sb = dtmpp.tile([128, 1024], BF16, tag="usb")
                          nc.scalar.activation(u_sb[:], pu[:], AF.Copy,
                                               bias=0.0, scale=1.0)
                          nc.vector.tensor_tensor(dtu2[:, sl], dt2[:, sl],
                                                  u_sb[:], OP.mult)

                      pb = pproj.tile([128, 1024], F32, tag="proj")
                      for hv in range(2):
                          cv = slice(512 * hv, 512 * hv + 512)
                          nc.tensor.matmul(pb[0:32, cv], lhsT=wseL[:, 512:544],
                                           rhs=ftL[:, cv], start=True, stop=False)
                          nc.tensor.matmul(pb[0:32, cv], lhsT=wseR[:, 544:576],
                                           rhs=ftR[:, cv], start=False, stop=True)
                      mview = umask[:, j * W:(j + 1) * W].unsqueeze(1) \
                          .broadcast_to((32, 8, W))
                      nc.vector.scalar_tensor_tensor(
                          bc[:, sl].rearrange("p (a b) -> p a b", b=W),
                          pb[0:32, :].rearrange("p (a b) -> p a b", b=W), 1.0,
                          mview, OP.mult, OP.mult)
                  return dt2, dtu2, bc

              def phase_pairs(st, prange, htiles, csm=None, _btm=None,
                              last=False):
                  dt2, dtu2, bc = st
                  if _btm is None:
                      _btm = _btmask
                  _bbq = nc.sync if int(os.environ.get("KERNEL_BBSP", "0")) else nc.gpsimd
                  if csm is None:
                      csm = csmp.tile([128, CCOLS], BF16)
                      _bbq.dma_start(
                          csm[:],
                          bc[16:24, :].unsqueeze(1).broadcast_to((8, 16, CCOLS)))
                  for p in prange:
                      bb = bbp.tile([128, CCOLS], BF16)
                      bt = bpl.tile([128, CCOLS], BF16)
                      _sliced = (ch == 0 and
                                 p < int(os.environ.get("KERNEL_WSLICE", "2"))
                                 ) or last or _SLALL
                      if _sliced:
                          for s4 in range(4):
                              sl = slice(s4 * 1024, s4 * 1024 + 1024)
                              _bbq.dma_start(
                                  bb[:, sl],
                                  bc[8 + 2 * p:8 + 2 * p + 2, sl].unsqueeze(1)
                                  .broadcast_to((2, 64, 1024)))
                      else:
                          _bbq.dma_start(
                              bb[:],
                              bc[8 + 2 * p:8 + 2 * p + 2, :].unsqueeze(1)
                              .broadcast_to((2, 64, CCOLS)))
                      av = apl.tile([128, CCOLS], BF16)
                      if last or _SLALL:
                          # per-s4 slices: 1024-col boundaries are row starts
                          # (av[w=0]=0 restarts the scan), so split ops are
                          # exact and range-tracked deps let the tail start
                          # after the first slice
                          for s4 in range(4):
                              sl = slice(s4 * 1024, s4 * 1024 + 1024)
                              nc.scalar.activation(
                                  av[:, sl], dt2[:, sl], AF.Exp,
                                  bias=0.0, scale=avec[:, 1 + p: 2 + p])
                              nc.vector.memset(
                                  av[:, sl].rearrange("p (h w) -> p h w", w=W)
                                  [:, :, 0:1], 0)
                      else:
                          nc.scalar.activation(av[:], dt2[:], AF.Exp,
                                               bias=0.0,
                                               scale=avec[:, 1 + p: 2 + p])
                          nc.vector.memset(
                              av[:].rearrange("p (h w) -> p h w", w=W)
                              [:, :, 0:1], 0)
                      bteng = nc.gpsimd if (_btm >> p) & 1 else nc.vector
                      if _sliced:
                          for s4 in range(4):
                              sl = slice(s4 * 1024, s4 * 1024 + 1024)
                              bteng.tensor_tensor(bt[:, sl], dtu2[:, sl],
                                                  bb[:, sl], OP.mult)
                      else:
                          bteng.tensor_tensor(bt[:], dtu2[:], bb[:], OP.mult)
                      hT = hpl.tile([128, CCOLS], BF16)
                      if last or _SLALL:
                          for s4 in range(4):
                              sl = slice(s4 * 1024, s4 * 1024 + 1024)
                              nc.vector.tensor_tensor_scan(
                                  hT[:, sl], av[:, sl], bt[:, sl], 0.0,
                                  OP.mult, OP.add)
                      else:
                          nc.vector.tensor_tensor_scan(hT[:], av[:], bt[:], 0.0,
                                                       OP.mult, OP.add)
                      htiles.append(hT)
                  return csm

              def phase_tail(ch, st, pr):
                  j, hh = divmod(ch, 2)
                  dt2, dtu2, bc = st
                  htiles, csm = pr
                  tt = tpl.tile([128, CCOLS], BF16, tag="tt")
                  for s8 in range(4):
                      sl10 = slice(s8 * 1024, s8 * 1024 + 1024)
                      zp = pz.tile([128, 1024], F32, tag="zp")
                      for half in range(2):
                          zv = slice(512 * half, 512 * half + 512)
                          sl5 = slice(s8 * 1024 + 512 * half,
                                      s8 * 1024 + 512 * half + 512)
                          for p in range(4):
                              nc.tensor.matmul(zp[32 * p:32 * p + 32, zv],
                                               lhsT=wbf[:, 0:32],
                                               rhs=htiles[p][:, sl5],
                                               start=True, stop=True,
                                               tile_position=(0, 32 * p))
                      _tta = int(os.environ.get("KERNEL_TTACT", "4"))
                      if ch == NCH - 1 and int(os.environ.get("KERNEL_T5", "1")):
                          _tta = 0
                      if s8 < _tta:
                          z_sb = epi.tile([128, 1024], BF16, tag="scr4k")
                          nc.scalar.activation(z_sb[:], zp[:], AF.Copy,
                                               bias=0.0, scale=1.0)
                          nc.vector.tensor_tensor(tt[:, sl10], z_sb[:],
                                                  csm[:, sl10], OP.mult)
                      else:
                          nc.vector.scalar_tensor_tensor(tt[:, sl10], zp[:], 1.0,
                                                         csm[:, sl10], OP.mult, OP.mult)

                  if int(os.environ.get("KERNEL_CSHARE", "0")):
                      cstg = tpl.tile([8, CCOLS], BF16, tag="tt")
                  else:
                      cstg = cstgp.tile([8, CCOLS], BF16)
                  for s4 in range(4):
                      sl = slice(s4 * 1024, s4 * 1024 + 1024)
                      cp = pc.tile([8, 1024], F32, tag="cp")
                      for hv in range(2):
                          cv = slice(512 * hv, 512 * hv + 512)
                          cg = slice(s4 * 1024 + 512 * hv, s4 * 1024 + 512 * hv + 512)
                          nc.tensor.matmul(cp[:, cv], lhsT=wbf[:, 32:40],
                                           rhs=tt[:, cg], start=True, stop=False)
                          nc.tensor.matmul(cp[:, cv], lhsT=wbf[0:8, 40:48],
                                           rhs=bc[0:8, cg], start=False, stop=True)
                      nc.scalar.activation(
                          cstg[:, sl], cp[:], AF.Copy, bias=0.0, scale=1.0,
                          accum_out=acc24[:, ch * 4 + s4: ch * 4 + s4 + 1])

                  row0 = j * 64 + hh * 32
                  st_t, st_r = (S1, row0) if row0 < 128 else (S2, row0 - 128)
                  _sgs = int(os.environ.get("KERNEL_SGSPLIT", "0")) or \
                      (ch == NCH - 1 and int(os.environ.get("KERNEL_SGLAST", "1")))
                  for g in range(G):
                      q = nc.gpsimd if (_sgs and g % 2) else nc.sync
                      q.dma_start(
                          st_t[st_r:st_r + 32, g * W:(g + 1) * W],
                          cstg[g:g + 1, :].rearrange("p (h w) -> p h w", w=W))

              from concourse import bass_isa
              rr = epi.tile([64, JD * G], BF16, tag="rr")
              ppool = epi.tile([G, 8], BF16, tag="ppool")
              arr = epi.tile([64, JD * G], BF16, tag="arr")

              def mx_path_s1():
                  sm1 = epi.tile([128, G * W], BF16, tag="sm1")
                  nc.vector.tensor_tensor(sm1[:], S1[:], mnegs[:, 0:G * W], OP.add)
                  r1 = epi.tile([128, G], BF16, tag="r1")
                  nc.vector.tensor_reduce(
                      r1[:], sm1[:].rearrange("p (g w) -> p g w", w=W),
                      AX.X, OP.max)
                  nc.gpsimd.dma_start(rr[:, 0:G], r1[0:64, :])
                  nc.sync.dma_start(rr[:, G:2 * G], r1[64:128, :])
                  nc.gpsimd.partition_all_reduce(
                      arr[:, 0:2 * G], rr[:, 0:2 * G], 64, bass_isa.ReduceOp.max)
                  nc.gpsimd.dma_start(ppool[:, 3:4], arr[0:1, 0:G])
                  nc.sync.dma_start(ppool[:, 4:5], arr[0:1, G:2 * G])

              def mx_path_s2():
                  sm2 = epi.tile([64, G * W], BF16, tag="sm2")
                  nc.vector.tensor_tensor(sm2[:], S2[:],
                                          mnegs[0:64, G * W:2 * G * W], OP.add)
                  r2 = epi.tile([64, G], BF16, tag="r2")
                  nc.vector.tensor_reduce(
                      r2[:], sm2[:].rearrange("p (g w) -> p g w", w=W),
                      AX.X, OP.max)
                  nc.gpsimd.partition_all_reduce(
                      arr[:, 2 * G:3 * G], r2[:], 64, bass_isa.ReduceOp.max)
                  # transpose [1,8] -> [8,1] on PE (outer product with 1.0)
                  # instead of a ~2.5us transposing DMA
                  pmx = pc.tile([8, 1], F32, tag="cp")
                  nc.tensor.matmul(pmx[:], lhsT=arr[0:1, 2 * G:3 * G],
                                   rhs=mlpw[0:1, 16:17], start=True, stop=True)
                  nc.vector.tensor_copy(ppool[:, 7:8], pmx[:])

              gb1 = epi.tile([128, 8], BF16, tag="gb1")
              gb2 = epi.tile([64, 8], BF16, tag="gb2")

              def epi01():
                  # channel attention for j0/j1: their chunks (0-3) are done
                  avgr01 = epi.tile([G, 2], F32, tag="avgr01")
                  nc.vector.tensor_reduce(
                      avgr01[:], acc24[:, 0:16].rearrange("p (j r) -> p j r", r=8),
                      AX.X, OP.add)
                  nc.vector.tensor_tensor(ppool[:, 0:2], avgr01[:],
                                          invc[:, 0:2], OP.mult)
                  z1p01 = pc.tile([4, 4], F32, tag="cp")
                  nc.tensor.matmul(z1p01[:, 0:2], lhsT=mlpw[:, 0:4],
                                   rhs=ppool[:, 0:2], start=True, stop=True)
                  nc.tensor.matmul(z1p01[:, 2:4], lhsT=mlpw[:, 0:4],
                                   rhs=ppool[:, 3:5], start=True, stop=True)
                  nc.scalar.activation(z1ones[0:4, 0:2], z1p01[:, 0:2], AF.Relu,
                                       bias=mlpw[0:4, 12:13], scale=1.0)
                  nc.scalar.activation(z1ones[0:4, 3:5], z1p01[:, 2:4], AF.Relu,
                                       bias=mlpw[0:4, 12:13], scale=1.0)
                  gp01 = pc.tile([2, 8], F32, tag="cp")
                  nc.tensor.matmul(gp01[:], lhsT=z1ones[0:5, 0:2],
                                   rhs=mlpw[0:5, 4:12], start=True, stop=False)
                  nc.tensor.matmul(gp01[:], lhsT=z1ones[0:5, 3:5],
                                   rhs=mlpw[0:5, 4:12], start=False, stop=True)
                  # sigmoid via exp + reciprocal: stays on act table 6
                  eg01 = epi.tile([2, 8], F32, tag="r1")
                  nc.scalar.activation(eg01[:], gp01[:], AF.Exp,
                                       bias=0.0, scale=-1.0)
                  egp01 = epi.tile([2, 8], F32, tag="r2")
                  nc.vector.tensor_scalar_add(egp01[:], eg01[:], 1.0)
                  chg01 = epi.tile([2, 8], BF16, tag="chg01")
                  with nc.allow_low_precision(reason="bf16 channel gate"):
                      nc.vector.reciprocal(chg01[:], egp01[:])
                  for jj, q in ((0, nc.gpsimd), (1, nc.sync)):
                      q.dma_start(
                          gb1[64 * jj:64 * jj + 64, :],
                          chg01[jj:jj + 1, :].unsqueeze(1)
                          .broadcast_to((1, 64, 8)))

              def s_gate(Sg, gb, rows, obase, tg):
                  Sgf = epi.tile([rows, G * W], BF16,
                                 tag="sm1" if rows == 128 else "sm2")
                  gview = gb[0:rows, :].unsqueeze(2).broadcast_to((rows, G, W))
                  nc.vector.tensor_tensor(
                      Sgf[:].rearrange("p (a b) -> p a b", b=W),
                      Sg[:].rearrange("p (a b) -> p a b", b=W), gview, OP.mult)
                  sv = Sgf[:].rearrange("p (g w) -> p w g", g=G)
                  ssum = epi.tile([rows, W], BF16, tag="ss" + tg)
                  smx = epi.tile([rows, W], BF16, tag="sm" + tg)
                  if tg == "b":
                      # pairwise trees at the DVE 2x TT rate instead of 1x
                      # reduces (flat column slices, distinct outputs only)
                      a4 = epi.tile([rows, 4 * W], BF16, tag="sm1")
                      a2 = epi.tile([rows, 2 * W], BF16, tag="scr4k")
                      with nc.allow_low_precision(reason="bf16 channel mean"):
                          nc.vector.tensor_tensor(
                              a4[:], Sgf[:, 0:4 * W], Sgf[:, 4 * W:8 * W],
                              OP.add)
                          nc.vector.tensor_tensor(
                              a2[:], a4[:, 0:2 * W], a4[:, 2 * W:4 * W], OP.add)
                          nc.vector.tensor_tensor(
                              ssum[:], a2[:, 0:W], a2[:, W:2 * W], OP.add)
                      m4 = epi.tile([rows, 4 * W], BF16, tag="sm1")
                      m2 = epi.tile([rows, 2 * W], BF16, tag="scr4k")
                      nc.vector.tensor_tensor(
                          m4[:], Sgf[:, 0:4 * W], Sgf[:, 4 * W:8 * W], OP.max)
                      nc.vector.tensor_tensor(
                          m2[:], m4[:, 0:2 * W], m4[:, 2 * W:4 * W], OP.max)
                      nc.vector.tensor_tensor(
                          smx[:], m2[:, 0:W], m2[:, W:2 * W], OP.max)
                  else:
                      with nc.allow_low_precision(reason="8-term channel mean"):
                          nc.vector.tensor_reduce(ssum[:], sv, AX.X, OP.add)
                      nc.vector.tensor_reduce(smx[:], sv, AX.X, OP.max)
                  q1 = epi.tile([rows, W], BF16, tag="q1" + tg)
                  nc.vector.tensor_scalar_mul(q1[:], smx[:], wsp[0:rows, 1:2])
                  gi = epi.tile([rows, W], BF16, tag="gi" + tg)
                  nc.vector.scalar_tensor_tensor(gi[:], ssum[:], wsp[0:rows, 0:1],
                                                 q1[:], OP.mult, OP.add)
                  # sigmoid(gi + b_sp) = 1/(1 + e^(-gi - b_sp)); wsp col3 = -b_sp
                  ei = epi.tile([rows, W], BF16, tag="sm" + tg)
                  nc.scalar.activation(ei[:], gi[:], AF.Exp,
                                       bias=wsp[0:rows, 3:4], scale=-1.0)
                  e1 = epi.tile([rows, W], BF16, tag="q1" + tg)
                  nc.vector.tensor_scalar_add(e1[:], ei[:], 1.0)
                  sg = epi.tile([rows, W], BF16, tag="gi" + tg)
                  with nc.allow_low_precision(reason="bf16 spatial gate"):
                      nc.vector.reciprocal(sg[:], e1[:])
                  O = epi.tile([rows, G * W], BF16, tag="scr4k")
                  oview = sg[:].unsqueeze(1).broadcast_to((rows, G, W))
                  nc.vector.tensor_tensor(
                      O[:].rearrange("p (a b) -> p a b", b=W),
                      Sgf[:].rearrange("p (a b) -> p a b", b=W), oview, OP.mult)
                  nc.gpsimd.dma_start(
                      out_d[obase:obase + rows].rearrange("r g w -> r (g w)"), O[:])

              # ---- software-pipelined chunk schedule ----
              # emission order: s4(k) | tail(k-1) | pairs01(k) | s4(k+1) |
              #                 pairs23(k) | tail(k) | ...
              prev = None
              pend = None       # (ch, st, htiles, csm) with pairs23 pending
              for ch in range(NCH):
                  st = phase_s4(ch)
                  if pend is not None:
                      phase_pairs(pend[1], range(2, 4), pend[2], pend[3],
                                  _btm=_btlist[pend[0]])
                      prev = (pend[0], pend[1], (pend[2], pend[3]))
                      pend = None
                  if prev is not None:
                      phase_tail(prev[0], prev[1], prev[2])
                      if prev[0] == 3:
                          mx_path_s1()
                          epi01()
                      if prev[0] == 4:
                          s_gate(S1, gb1, 128, 0, "a")
                      prev = None
                  htiles = []
                  csm = phase_pairs(st, range(0, 2), htiles, _btm=_btlist[ch],
                                    last=(ch == NCH - 1 and _SL01))
                  pend = (ch, st, htiles, csm)
              phase_pairs(pend[1], range(2, 4), pend[2], pend[3],
                          _btm=_btlist[pend[0]], last=True)
              phase_tail(pend[0], pend[1], (pend[2], pend[3]))
              mx_path_s2()

              # ---------- j2 epilogue (only piece left after last tail) ----
              avgr2 = epi.tile([G, 1], F32, tag="avgr2")
              nc.vector.tensor_reduce(avgr2[:], acc24[:, 16:24], AX.X, OP.add)
              nc.vector.tensor_tensor(ppool[:, 6:7], avgr2[:], invc[:, 2:3],
                                      OP.mult)
              z1p2 = pc.tile([4, 2], F32, tag="cp")
              nc.tensor.matmul(z1p2[:], lhsT=mlpw[:, 0:4], rhs=ppool[:, 6:8],
                               start=True, stop=True)
              nc.scalar.activation(z1ones[0:4, 2:3], z1p2[:, 0:1], AF.Relu,
                                   bias=mlpw[0:4, 12:13], scale=1.0)
              nc.scalar.activation(z1ones[0:4, 5:6], z1p2[:, 1:2], AF.Relu,
                                   bias=mlpw[0:4, 12:13], scale=1.0)
              gp2 = pc.tile([1, 8], F32, tag="cp")
              nc.tensor.matmul(gp2[:], lhsT=z1ones[0:5, 2:3], rhs=mlpw[0:5, 4:12],
                               start=True, stop=False)
              nc.tensor.matmul(gp2[:], lhsT=z1ones[0:5, 5:6], rhs=mlpw[0:5, 4:12],
                               start=False, stop=True)
              eg2 = epi.tile([1, 8], F32, tag="r1")
              nc.scalar.activation(eg2[:], gp2[:], AF.Exp, bias=0.0, scale=-1.0)
              egp2 = epi.tile([1, 8], F32, tag="avgr01")
              nc.vector.tensor_scalar_add(egp2[:], eg2[:], 1.0)
              chg2 = epi.tile([1, 8], BF16, tag="chg2")
              with nc.allow_low_precision(reason="bf16 channel gate"):
                  nc.vector.reciprocal(chg2[:], egp2[:])
              nc.gpsimd.partition_broadcast(gb2[:], chg2[:], channels=64)
              s_gate(S2, gb2, 64, 128, "b")
            for _it in range(_ITERS):
                _one_iter()

    nc.compile()
    return nc


def _host_inputs(inputs):
    """Build the 8 per-core input maps from the full problem inputs."""
    import ml_dtypes
    L = _f32(inputs["featuresL"])[0]          # [C,H,W]
    R = _f32(inputs["featuresR"])[0]
    W_in = _f32(inputs["W_in"])
    W_dt = _f32(inputs["W_dt"])
    b_dt = _f32(inputs["b_dt"])
    W_B = _f32(inputs["W_B"])
    W_C = _f32(inputs["W_C"])
    A = -np.exp(_f32(inputs["A_log"]))        # [E,S]
    D_skip = _f32(inputs["D_skip"])
    W_out = _f32(inputs["W_out"])
    W1, b1 = _f32(inputs["W1"]), _f32(inputs["b1"])
    W2, b2 = _f32(inputs["W2"]), _f32(inputs["b2"])
    w_sp, b_sp = _f32(inputs["w_sp"]), _f32(inputs["b_sp"])

    # stationary weights [64, 576]
    idx = np.arange(128) % 64
    wse = np.zeros((2 * C, 576), np.float32)
    wse[0:32, 0:128] = W_in[0::2][:, idx]
    wse[32:64, 128:256] = W_in[1::2][:, idx]
    wse[0:32, 256:384] = W_dt[0::2][:, idx]
    wse[32:64, 384:512] = W_dt[1::2][:, idx]
    W_comb = W_in @ (D_skip[:, None] * W_out)        # [64(c), G]
    wse[0:32, 512:520] = W_comb[0::2]
    wse[0:32, 520:528] = W_B[0::2]
    wse[0:32, 528:536] = W_C[0::2]
    wse[32:64, 544:552] = W_comb[1::2]
    wse[32:64, 552:560] = W_B[1::2]
    wse[32:64, 560:568] = W_C[1::2]

    # bf16 stationaries [128, 48]
    wbf = np.zeros((128, 48), np.float32)
    for row in range(128):
        cc, e = divmod(row, 64)
        for q in range(32):
            c2, g = q // 16, q % 16
            if g < 8 and cc == c2:
                wbf[row, q] = W_out[e, g]
    for p4 in range(4):
        for local in range(32):
            c2, g = local // 16, local % 16
            if g < 8:
                wbf[32 * p4 + local, 32 + g] = 1.0
    wbf[0:8, 40:48] = np.eye(8, dtype=np.float32)

    avec = np.zeros((128, 8), np.float32)
    avec[:, 0] = b_dt[idx]
    for p4 in range(4):
        cc = np.arange(128) // 64
        avec[:, 1 + p4] = A[idx, 2 * p4 + cc]

    wspv = np.zeros((128, 4), np.float32)
    wspv[:, 0] = w_sp[0] / G
    wspv[:, 1] = w_sp[1]
    wspv[:, 2] = np.float32(np.asarray(b_sp).reshape(-1)[0]) if np.asarray(b_sp).size else 0.0
    wspv[:, 3] = -wspv[:, 2]

    mlpv = np.zeros((G, 24), np.float32)
    mlpv[:, 0:4] = W1
    mlpv[0:4, 4:12] = W2
    mlpv[4, 4:12] = 2.0 * b2
    mlpv[0:4, 12] = b1
    mlpv[0, 16:19] = 1.0

    maps = []
    wi = np.arange(W)
    for k in range(NCORES):
        d0 = JD * k
        Rsh = np.zeros_like(R)
        if d0 > 0:
            Rsh[:, :, d0:] = R[:, :, :-d0]
        else:
            Rsh = R
        feat = np.zeros((C, 2 * FROW), np.float32)
        feat[:, PAD:PAD + HW] = L.reshape(C, HW)
        feat[:, FROW + PAD:] = Rsh.reshape(C, HW)

        umask = np.zeros((32, JD * W), np.float32)
        for j in range(JD):
            umask[:, j * W:(j + 1) * W] = (wi >= d0 + j).astype(np.float32)[None]

        # S-layout max-pool masks: S1 rows (j=0,1), S2 rows (j=2)
        mnegs = np.full((128, 2 * G * W), -1e30, np.float32)
        for j in range(2):
            row_mask = np.where(wi >= d0 + j, 0.0, -1e30)          # [W]
            mnegs[j * 64:(j + 1) * 64, 0:G * W] = np.tile(row_mask, G)[None]
        mnegs[0:64, G * W:2 * G * W] = np.tile(
            np.where(wi >= d0 + 2, 0.0, -1e30), G)[None]

        invc = np.zeros((G, JD), np.float32)
        for j in range(JD):
            invc[:, j] = 1.0 / (H * (W - (d0 + j)))

        maps.append({
            "feat": feat.astype(ml_dtypes.bfloat16),
            "wse": wse.astype(ml_dtypes.bfloat16),
            "wbf": wbf.astype(ml_dtypes.bfloat16),
            "avec": avec,
            "umask": umask.astype(ml_dtypes.bfloat16),
            "mnegs": mnegs.astype(ml_dtypes.bfloat16),
            "invc": invc,
            "wsp": wspv,
            "mlp": mlpv.astype(ml_dtypes.bfloat16),
        })
    return maps


def kernel(**inputs):
    from concourse.bass_utils import run_bass_kernel_spmd

    if "nc" not in _compiled:
        _compiled["nc"] = _build_program()
    nc = _compiled["nc"]

    maps = _host_inputs(inputs)
    res = run_bass_kernel_spmd(nc, maps, list(range(NCORES))).results

    vol = np.zeros((1, G, DV, H, W), np.float32)
    for k in range(NCORES):
        o = np.asarray(res[k]["out"], np.float32).reshape(JD, H, G, W)        # [j,h,g,w]
        vol[0, :, JD * k:JD * k + JD] = np.transpose(o, (2, 0, 1, 3))
    return vol



# revision 45
# speedup vs baseline: 1.0288x; 1.0065x over previous
"""Trainium2 Bass kernel for nn_BuildCostVolume (stereo cost volume + Mamba scan).

Sharding: disparity axis (24) split as 3 per core across 8 cores; core k
handles disparities d = 3k+j (j in 0..2, compile-time; host pre-shifts
featuresR by 3k so the SPMD program is identical across cores).

Per-core pipeline (software-pipelined across the 6 (j, h-half) chunks as
s4(k) | tail(k-1) | pairs01(k) | s4(k+1) | pairs23(k) | tail(k) | ...):
  - Features loaded once as bf16; u/dt/B/C/D projections on PE from L and
    shifted-R views with even/odd split weights (channel interleave trick).
  - dt = softplus via Exp + Ln(x+1) on ACT; u evicted via ACT Copy so the
    dt*u multiply runs at the DVE 2x (16-bit) rate.
  - Decay a = exp(A*dt) via ACT per-partition scale in an (s-pair x e)
    128-partition layout; B broadcast via SBUF-to-SBUF DMA (Pool queue);
    b = dt*u*B on DVE, with pairs 1,3 offloaded to GPSIMD to balance.
  - Mamba recurrence h = a*h + b via DVE tensor_tensor_scan over flattened
    (row, w) with a[w=0]=0 so each image row restarts the scan.
  - y/cost contraction on PE (block-diag W_out fold, C multiply at PSUM
    eviction, partition-sum + D-term matmul); cstg evicted on ACT with
    fused avg-pool accumulation.
  - Channel-attn max pool from the spatial S layout: masked add + per-g
    max on 128 partitions, GPSIMD partition_all_reduce, tiny transposing
    DMAs; MLP in bf16; spatial attention as in the reference.
  - Output written [j*64+h, g, w] bf16 and transposed/cast on host.
"""
import os
import numpy as np

C, H, W, DV = 32, 64, 128, 24
_NCH_ENV = int(os.environ.get("KERNEL_NCH", "6"))
_SKIP_EPI = bool(int(os.environ.get("KERNEL_SKIP_EPI", "0")))
_SKIP_PAIRS = bool(int(os.environ.get("KERNEL_SKIP_PAIRS", "0")))
_ITERS = int(os.environ.get("KERNEL_ITERS", "1"))
_SL01 = bool(int(os.environ.get("KERNEL_SL01", "1")))
_SLALL = bool(int(os.environ.get("KERNEL_SLALL", "0")))
_SL0 = bool(int(os.environ.get("KERNEL_SL0", "1")))
E, S, G = 64, 8, 8
NCORES, JD = 8, 3          # cores, disparities per core
PAD = 8                    # leading zero columns in feature tensors
HH = 32                    # h rows per chunk
NCH = 6                    # chunks = (j, h-half)
CCOLS = HH * W             # 4096 columns per chunk
HW = H * W                 # 8192
FROW = PAD + HW            # 8200 cols per feature image

_compiled = {}


def _f32(x):
    return np.ascontiguousarray(np.asarray(x, np.float32))


def _build_program():
    import concourse.bacc as bacc
    import concourse.mybir as mybir
    from concourse.tile import TileContext

    F32 = mybir.dt.float32
    BF16 = mybir.dt.bfloat16
    AF = mybir.ActivationFunctionType
    AX = mybir.AxisListType
    OP = mybir.AluOpType

    nc = bacc.Bacc("TRN2", target_bir_lowering=False, debug=False,
                   num_devices=NCORES)

    feat_d = nc.dram_tensor("feat", [C, 2 * FROW], BF16, kind="ExternalInput").ap()
    wse_d = nc.dram_tensor("wse", [2 * C, 576], BF16, kind="ExternalInput").ap()
    wbf_d = nc.dram_tensor("wbf", [128, 48], BF16, kind="ExternalInput").ap()
    avec_d = nc.dram_tensor("avec", [128, 8], F32, kind="ExternalInput").ap()
    umask_d = nc.dram_tensor("umask", [32, JD * W], BF16, kind="ExternalInput").ap()
    mnegs_d = nc.dram_tensor("mnegs", [128, 2 * G * W], BF16, kind="ExternalInput").ap()
    invc_d = nc.dram_tensor("invc", [G, JD], F32, kind="ExternalInput").ap()
    wsp_d = nc.dram_tensor("wsp", [128, 4], F32, kind="ExternalInput").ap()
    mlp_d = nc.dram_tensor("mlp", [G, 24], BF16, kind="ExternalInput").ap()
    out_d = nc.dram_tensor("out", [JD * H, G, W], BF16, kind="ExternalOutput").ap()

    with TileContext(nc) as tc:
        with tc.tile_pool(name="const", bufs=1) as cpool, \
             tc.tile_pool(name="dtmp", bufs=1) as dtmpp, \
             tc.tile_pool(name="dt2", bufs=2) as dt2p, \
             tc.tile_pool(name="dtu2", bufs=2) as dtu2p, \
             tc.tile_pool(name="bc", bufs=2) as bcp, \
             tc.tile_pool(name="bb", bufs=2) as bbp, \
             tc.tile_pool(name="csm", bufs=1) as csmp, \
             tc.tile_pool(name="apool", bufs=int(os.environ.get("KERNEL_AB", "2"))) as apl, \
             tc.tile_pool(name="bpool", bufs=int(os.environ.get("KERNEL_BB", "2"))) as bpl, \
             tc.tile_pool(name="hpool", bufs=int(os.environ.get("KERNEL_HB", "4"))) as hpl, \
             tc.tile_pool(name="tpool", bufs=1) as tpl, \
             tc.tile_pool(name="cstg", bufs=1) as cstgp, \
             tc.tile_pool(name="epi", bufs=1) as epi, \
             tc.tile_pool(name="pproj", bufs=2, space="PSUM") as pproj, \
             tc.tile_pool(name="pz", bufs=1, space="PSUM") as pz, \
             tc.tile_pool(name="pc", bufs=1, space="PSUM") as pc:

            _ld = mybir.InstLoadActFuncSet(
                name=nc.get_next_instruction_name(), act_func_set_id=6,
                ins=[], outs=[])
            nc.scalar.add_instruction(_ld)
            wseL = cpool.tile([C, 576], BF16)
            nc.sync.dma_start(wseL[:], wse_d[0:C, :])
            wseR = cpool.tile([C, 576], BF16)
            nc.sync.dma_start(wseR[:], wse_d[C:2 * C, :])
            avec = cpool.tile([128, 8], F32)
            nc.sync.dma_start(avec[:], avec_d[:])
            featsb = cpool.tile([C, 2 * FROW], BF16)
            HB = PAD + HH * W
            nc.sync.dma_start(featsb[:, 0:HB], feat_d[:, 0:HB])
            nc.gpsimd.dma_start(featsb[:, FROW:FROW + HB],
                                feat_d[:, FROW:FROW + HB])
            nc.sync.dma_start(featsb[:, HB:FROW], feat_d[:, HB:FROW])
            nc.gpsimd.dma_start(featsb[:, FROW + HB:2 * FROW],
                                feat_d[:, FROW + HB:2 * FROW])
            wbf = cpool.tile([128, 48], BF16)
            nc.sync.dma_start(wbf[:], wbf_d[:])
            umask = cpool.tile([32, JD * W], BF16)
            nc.sync.dma_start(umask[:], umask_d[:])
            mnegs = cpool.tile([128, 2 * G * W], BF16)
            nc.sync.dma_start(mnegs[:], mnegs_d[:])
            invc = cpool.tile([G, JD], F32)
            nc.sync.dma_start(invc[:], invc_d[:])
            wsp = cpool.tile([128, 4], F32)
            nc.sync.dma_start(wsp[:], wsp_d[:])
            mlpw = cpool.tile([G, 24], BF16)
            nc.sync.dma_start(mlpw[:], mlp_d[:])
            z1ones = cpool.tile([5, 8], BF16)
            nc.sync.dma_start(z1ones[4:5, 0:6], mlpw[0:1, 16:22])

            def _one_iter():
              acc24 = epi.tile([G, 24], F32, tag="acc24")    # per-(chunk,s4) sums
              S1 = epi.tile([128, G * W], BF16, tag="S1")    # spatial rows 0-127
              S2 = epi.tile([64, G * W], BF16, tag="S2")     # spatial rows 128-191
              _btmask = int(os.environ.get("KERNEL_BTPOOL", "10"))
              _btlist = [int(x) for x in os.environ.get(
                  "KERNEL_BTLIST", "10,10,10,10,10,10").split(",")]

              def phase_s4(ch):
                  j, hh = divmod(ch, 2)
                  base = hh * CCOLS
                  dt2 = dt2p.tile([128, CCOLS], BF16)
                  dtu2 = dtu2p.tile([128, CCOLS], BF16)
                  bc = bcp.tile([32, CCOLS], BF16)
                  for s4 in range(4):
                      cs = base + s4 * 1024
                      sl = slice(s4 * 1024, s4 * 1024 + 1024)
                      ftL = featsb[:, PAD + cs: PAD + cs + 1024]
                      ftR = featsb[:, FROW + PAD + cs - j: FROW + PAD + cs - j + 1024]

                      pd = pproj.tile([128, 1024], F32, tag="proj")
                      for hv in range(2):
                          cv = slice(512 * hv, 512 * hv + 512)
                          nc.tensor.matmul(pd[:, cv], lhsT=wseL[:, 256:384],
                                           rhs=ftL[:, cv], start=True, stop=False)
                          nc.tensor.matmul(pd[:, cv], lhsT=wseR[:, 384:512],
                                           rhs=ftR[:, cv], start=False, stop=True)
                      dm = dtmpp.tile([128, 1024], BF16, tag="dm")
                      nc.scalar.activation(dm[:], pd[:], AF.Exp,
                                           bias=avec[:, 0:1], scale=1.0)
                      nc.scalar.activation(dt2[:, sl], dm[:], AF.Ln, bias=1.0,
                                           scale=1.0)

                      pu = pproj.tile([128, 1024], F32, tag="proj")
                      for hv in range(2):
                          cv = slice(512 * hv, 512 * hv + 512)
                          nc.tensor.matmul(pu[:, cv], lhsT=wseL[:, 0:128],
                                           rhs=ftL[:, cv], start=True, stop=False)
                          nc.tensor.matmul(pu[:, cv], lhsT=wseR[:, 128:256],
                                           rhs=ftR[:, cv], start=False, stop=True)
                      if ch == 0 and int(os.environ.get("KERNEL_W0", "1")):
                          nc.vector.tensor_tensor(dtu2[:, sl], dt2[:, sl],
                                                  pu[:], OP.mult)
                      else:
                          u_sb = dtmpp.tile([128, 1024], BF16, tag="usb")
                          nc.scalar.activation(u_sb[:], pu[:], AF.Copy,
                                               bias=0.0, scale=1.0)
                          nc.vector.tensor_tensor(dtu2[:, sl], dt2[:, sl],
                                                  u_sb[:], OP.mult)

                      pb = pproj.tile([128, 1024], F32, tag="proj")
                      for hv in range(2):
                          cv = slice(512 * hv, 512 * hv + 512)
                          nc.tensor.matmul(pb[0:32, cv], lhsT=wseL[:, 512:544],
                                           rhs=ftL[:, cv], start=True, stop=False)
                          nc.tensor.matmul(pb[0:32, cv], lhsT=wseR[:, 544:576],
                                           rhs=ftR[:, cv], start=False, stop=True)
                      mview = umask[:, j * W:(j + 1) * W].unsqueeze(1) \
                          .broadcast_to((32, 8, W))
                      nc.vector.scalar_tensor_tensor(
                          bc[:, sl].rearrange("p (a b) -> p a b", b=W),
                          pb[0:32, :].rearrange("p (a b) -> p a b", b=W), 1.0,
                          mview, OP.mult, OP.mult)
                  return dt2, dtu2, bc

              def phase_pairs(st, prange, htiles, csm=None, _btm=None,
                              last=False):
                  dt2, dtu2, bc = st
                  if _btm is None:
                      _btm = _btmask
                  _bbq = nc.sync if int(os.environ.get("KERNEL_BBSP", "0")) else nc.gpsimd
                  if csm is None:
                      csm = csmp.tile([128, CCOLS], BF16)
                      _bbq.dma_start(
                          csm[:],
                          bc[16:24, :].unsqueeze(1).broadcast_to((8, 16, CCOLS)))
                  for p in prange:
                      bb = bbp.tile([128, CCOLS], BF16)
                      bt = bpl.tile([128, CCOLS], BF16)
                      _sliced = (ch == 0 and
                                 p < int(os.environ.get("KERNEL_WSLICE", "2"))
                                 ) or last or _SLALL
                      if _sliced:
                          for s4 in range(4):
                              sl = slice(s4 * 1024, s4 * 1024 + 1024)
                              _bbq.dma_start(
                                  bb[:, sl],
                                  bc[8 + 2 * p:8 + 2 * p + 2, sl].unsqueeze(1)
                                  .broadcast_to((2, 64, 1024)))
                      else:
                          _bbq.dma_start(
                              bb[:],
                              bc[8 + 2 * p:8 + 2 * p + 2, :].unsqueeze(1)
                              .broadcast_to((2, 64, CCOLS)))
                      av = apl.tile([128, CCOLS], BF16)
                      if last or _SLALL:
                          # per-s4 slices: 1024-col boundaries are row starts
                          # (av[w=0]=0 restarts the scan), so split ops are
                          # exact and range-tracked deps let the tail start
                          # after the first slice
                          for s4 in range(4):
                              sl = slice(s4 * 1024, s4 * 1024 + 1024)
                              nc.scalar.activation(
                                  av[:, sl], dt2[:, sl], AF.Exp,
                                  bias=0.0, scale=avec[:, 1 + p: 2 + p])
                              nc.vector.memset(
                                  av[:, sl].rearrange("p (h w) -> p h w", w=W)
                                  [:, :, 0:1], 0)
                      else:
                          nc.scalar.activation(av[:], dt2[:], AF.Exp,
                                               bias=0.0,
                                               scale=avec[:, 1 + p: 2 + p])
                          nc.vector.memset(
                              av[:].rearrange("p (h w) -> p h w", w=W)
                              [:, :, 0:1], 0)
                      bteng = nc.gpsimd if (_btm >> p) & 1 else nc.vector
                      if _sliced:
                          for s4 in range(4):
                              sl = slice(s4 * 1024, s4 * 1024 + 1024)
                              bteng.tensor_tensor(bt[:, sl], dtu2[:, sl],
                                                  bb[:, sl], OP.mult)
                      else:
                          bteng.tensor_tensor(bt[:], dtu2[:], bb[:], OP.mult)
                      hT = hpl.tile([128, CCOLS], BF16)
                      if last or _SLALL:
                          for s4 in range(4):
                              sl = slice(s4 * 1024, s4 * 1024 + 1024)
                              nc.vector.tensor_tensor_scan(
                                  hT[:, sl], av[:, sl], bt[:, sl], 0.0,
                                  OP.mult, OP.add)
                      else:
                          nc.vector.tensor_tensor_scan(hT[:], av[:], bt[:], 0.0,
                                                       OP.mult, OP.add)
                      htiles.append(hT)
                  return csm

              def phase_tail(ch, st, pr):
                  j, hh = divmod(ch, 2)
                  dt2, dtu2, bc = st
                  htiles, csm = pr
                  tt = tpl.tile([128, CCOLS], BF16, tag="tt")
                  for s8 in range(4):
                      sl10 = slice(s8 * 1024, s8 * 1024 + 1024)
                      zp = pz.tile([128, 1024], F32, tag="zp")
                      for half in range(2):
                          zv = slice(512 * half, 512 * half + 512)
                          sl5 = slice(s8 * 1024 + 512 * half,
                                      s8 * 1024 + 512 * half + 512)
                          for p in range(4):
                              nc.tensor.matmul(zp[32 * p:32 * p + 32, zv],
                                               lhsT=wbf[:, 0:32],
                                               rhs=htiles[p][:, sl5],
                                               start=True, stop=True,
                                               tile_position=(0, 32 * p))
                      _tta = int(os.environ.get("KERNEL_TTACT", "4"))
                      if ch == NCH - 1 and int(os.environ.get("KERNEL_T5", "1")):
                          _tta = 0
                      if s8 < _tta:
                          z_sb = epi.tile([128, 1024], BF16, tag="scr4k")
                          nc.scalar.activation(z_sb[:], zp[:], AF.Copy,
                                               bias=0.0, scale=1.0)
                          nc.vector.tensor_tensor(tt[:, sl10], z_sb[:],
                                                  csm[:, sl10], OP.mult)
                      else:
                          nc.vector.scalar_tensor_tensor(tt[:, sl10], zp[:], 1.0,
                                                         csm[:, sl10], OP.mult, OP.mult)

                  if int(os.environ.get("KERNEL_CSHARE", "0")):
                      cstg = tpl.tile([8, CCOLS], BF16, tag="tt")
                  else:
                      cstg = cstgp.tile([8, CCOLS], BF16)
                  for s4 in range(4):
                      sl = slice(s4 * 1024, s4 * 1024 + 1024)
                      cp = pc.tile([8, 1024], F32, tag="cp")
                      for hv in range(2):
                          cv = slice(512 * hv, 512 * hv + 512)
                          cg = slice(s4 * 1024 + 512 * hv, s4 * 1024 + 512 * hv + 512)
                          nc.tensor.matmul(cp[:, cv], lhsT=wbf[:, 32:40],
                                           rhs=tt[:, cg], start=True, stop=False)
                          nc.tensor.matmul(cp[:, cv], lhsT=wbf[0:8, 40:48],
                                           rhs=bc[0:8, cg], start=False, stop=True)
                      nc.scalar.activation(
                          cstg[:, sl], cp[:], AF.Copy, bias=0.0, scale=1.0,
                          accum_out=acc24[:, ch * 4 + s4: ch * 4 + s4 + 1])

                  row0 = j * 64 + hh * 32
                  st_t, st_r = (S1, row0) if row0 < 128 else (S2, row0 - 128)
                  _sgs = int(os.environ.get("KERNEL_SGSPLIT", "0")) or \
                      (ch == NCH - 1 and int(os.environ.get("KERNEL_SGLAST", "1")))
                  for g in range(G):
                      q = nc.gpsimd if (_sgs and g % 2) else nc.sync
                      q.dma_start(
                          st_t[st_r:st_r + 32, g * W:(g + 1) * W],
                          cstg[g:g + 1, :].rearrange("p (h w) -> p h w", w=W))

              from concourse import bass_isa
              rr = epi.tile([64, JD * G], BF16, tag="rr")
              ppool = epi.tile([G, 8], BF16, tag="ppool")
              arr = epi.tile([64, JD * G], BF16, tag="arr")

              def mx_path_s1():
                  sm1 = epi.tile([128, G * W], BF16, tag="sm1")
                  nc.vector.tensor_tensor(sm1[:], S1[:], mnegs[:, 0:G * W], OP.add)
                  r1 = epi.tile([128, G], BF16, tag="r1")
                  nc.vector.tensor_reduce(
                      r1[:], sm1[:].rearrange("p (g w) -> p g w", w=W),
                      AX.X, OP.max)
                  nc.gpsimd.dma_start(rr[:, 0:G], r1[0:64, :])
                  nc.sync.dma_start(rr[:, G:2 * G], r1[64:128, :])
                  nc.gpsimd.partition_all_reduce(
                      arr[:, 0:2 * G], rr[:, 0:2 * G], 64, bass_isa.ReduceOp.max)
                  nc.gpsimd.dma_start(ppool[:, 3:4], arr[0:1, 0:G])
                  nc.sync.dma_start(ppool[:, 4:5], arr[0:1, G:2 * G])

              def mx_path_s2():
                  sm2 = epi.tile([64, G * W], BF16, tag="sm2")
                  nc.vector.tensor_tensor(sm2[:], S2[:],
                                          mnegs[0:64, G * W:2 * G * W], OP.add)
                  r2 = epi.tile([64, G], BF16, tag="r2")
                  nc.vector.tensor_reduce(
                      r2[:], sm2[:].rearrange("p (g w) -> p g w", w=W),
                      AX.X, OP.max)
                  nc.gpsimd.partition_all_reduce(
                      arr[:, 2 * G:3 * G], r2[:], 64, bass_isa.ReduceOp.max)
                  # transpose [1,8] -> [8,1] on PE (outer product with 1.0)
                  # instead of a ~2.5us transposing DMA
                  pmx = pc.tile([8, 1], F32, tag="cp")
                  nc.tensor.matmul(pmx[:], lhsT=arr[0:1, 2 * G:3 * G],
                                   rhs=mlpw[0:1, 16:17], start=True, stop=True)
                  nc.vector.tensor_copy(ppool[:, 7:8], pmx[:])

              gb1 = epi.tile([128, 8], BF16, tag="gb1")
              gb2 = epi.tile([64, 8], BF16, tag="gb2")

              def epi01():
                  # channel attention for j0/j1: their chunks (0-3) are done
                  avgr01 = epi.tile([G, 2], F32, tag="avgr01")
                  nc.vector.tensor_reduce(
                      avgr01[:], acc24[:, 0:16].rearrange("p (j r) -> p j r", r=8),
                      AX.X, OP.add)
                  nc.vector.tensor_tensor(ppool[:, 0:2], avgr01[:],
                                          invc[:, 0:2], OP.mult)
                  z1p01 = pc.tile([4, 4], F32, tag="cp")
                  nc.tensor.matmul(z1p01[:, 0:2], lhsT=mlpw[:, 0:4],
                                   rhs=ppool[:, 0:2], start=True, stop=True)
                  nc.tensor.matmul(z1p01[:, 2:4], lhsT=mlpw[:, 0:4],
                                   rhs=ppool[:, 3:5], start=True, stop=True)
                  nc.scalar.activation(z1ones[0:4, 0:2], z1p01[:, 0:2], AF.Relu,
                                       bias=mlpw[0:4, 12:13], scale=1.0)
                  nc.scalar.activation(z1ones[0:4, 3:5], z1p01[:, 2:4], AF.Relu,
                                       bias=mlpw[0:4, 12:13], scale=1.0)
                  gp01 = pc.tile([2, 8], F32, tag="cp")
                  nc.tensor.matmul(gp01[:], lhsT=z1ones[0:5, 0:2],
                                   rhs=mlpw[0:5, 4:12], start=True, stop=False)
                  nc.tensor.matmul(gp01[:], lhsT=z1ones[0:5, 3:5],
                                   rhs=mlpw[0:5, 4:12], start=False, stop=True)
                  # sigmoid via exp + reciprocal: stays on act table 6
                  eg01 = epi.tile([2, 8], F32, tag="r1")
                  nc.scalar.activation(eg01[:], gp01[:], AF.Exp,
                                       bias=0.0, scale=-1.0)
                  egp01 = epi.tile([2, 8], F32, tag="r2")
                  nc.vector.tensor_scalar_add(egp01[:], eg01[:], 1.0)
                  chg01 = epi.tile([2, 8], BF16, tag="chg01")
                  with nc.allow_low_precision(reason="bf16 channel gate"):
                      nc.vector.reciprocal(chg01[:], egp01[:])
                  for jj, q in ((0, nc.gpsimd), (1, nc.sync)):
                      q.dma_start(
                          gb1[64 * jj:64 * jj + 64, :],
                          chg01[jj:jj + 1, :].unsqueeze(1)
                          .broadcast_to((1, 64, 8)))

              def s_gate(Sg, gb, rows, obase, tg):
                  Sgf = epi.tile([rows, G * W], BF16,
                                 tag="sm1" if rows == 128 else "sm2")
                  gview = gb[0:rows, :].unsqueeze(2).broadcast_to((rows, G, W))
                  nc.vector.tensor_tensor(
                      Sgf[:].rearrange("p (a b) -> p a b", b=W),
                      Sg[:].rearrange("p (a b) -> p a b", b=W), gview, OP.mult)
                  sv = Sgf[:].rearrange("p (g w) -> p w g", g=G)
                  ssum = epi.tile([rows, W], BF16, tag="ss" + tg)
                  smx = epi.tile([rows, W], BF16, tag="sm" + tg)
                  if tg == "b":
                      # pairwise trees at the DVE 2x TT rate instead of 1x
                      # reduces (flat column slices, distinct outputs only)
                      a4 = epi.tile([rows, 4 * W], BF16, tag="sm1")
                      a2 = epi.tile([rows, 2 * W], BF16, tag="scr4k")
                      with nc.allow_low_precision(reason="bf16 channel mean"):
                          nc.vector.tensor_tensor(
                              a4[:], Sgf[:, 0:4 * W], Sgf[:, 4 * W:8 * W],
                              OP.add)
                          nc.vector.tensor_tensor(
                              a2[:], a4[:, 0:2 * W], a4[:, 2 * W:4 * W], OP.add)
                          nc.vector.tensor_tensor(
                              ssum[:], a2[:, 0:W], a2[:, W:2 * W], OP.add)
                      m4 = epi.tile([rows, 4 * W], BF16, tag="sm1")
                      m2 = epi.tile([rows, 2 * W], BF16, tag="scr4k")
                      nc.vector.tensor_tensor(
                          m4[:], Sgf[:, 0:4 * W], Sgf[:, 4 * W:8 * W], OP.max)
                      nc.vector.tensor_tensor(
                          m2[:], m4[:, 0:2 * W], m4[:, 2 * W:4 * W], OP.max)
                      nc.vector.tensor_tensor(
                          smx[:], m2[:, 0:W], m2[:, W:2 * W], OP.max)
                  else:
                      with nc.allow_low_precision(reason="8-term channel mean"):
                          nc.vector.tensor_reduce(ssum[:], sv, AX.X, OP.add)
                      nc.vector.tensor_reduce(smx[:], sv, AX.X, OP.max)
                  q1 = epi.tile([rows, W], BF16, tag="q1" + tg)
                  nc.vector.tensor_scalar_mul(q1[:], smx[:], wsp[0:rows, 1:2])
                  gi = epi.tile([rows, W], BF16, tag="gi" + tg)
                  nc.vector.scalar_tensor_tensor(gi[:], ssum[:], wsp[0:rows, 0:1],
                                                 q1[:], OP.mult, OP.add)
                  # sigmoid(gi + b_sp) = 1/(1 + e^(-gi - b_sp)); wsp col3 = -b_sp
                  ei = epi.tile([rows, W], BF16, tag="sm" + tg)
                  nc.scalar.activation(ei[:], gi[:], AF.Exp,
                                       bias=wsp[0:rows, 3:4], scale=-1.0)
                  e1 = epi.tile([rows, W], BF16, tag="q1" + tg)
                  nc.vector.tensor_scalar_add(e1[:], ei[:], 1.0)
                  sg = epi.tile([rows, W], BF16, tag="gi" + tg)
                  with nc.allow_low_precision(reason="bf16 spatial gate"):
                      nc.vector.reciprocal(sg[:], e1[:])
                  O = epi.tile([rows, G * W], BF16, tag="scr4k")
                  oview = sg[:].unsqueeze(1).broadcast_to((rows, G, W))
                  nc.vector.tensor_tensor(
                      O[:].rearrange("p (a b) -> p a b", b=W),
                      Sgf[:].rearrange("p (a b) -> p a b", b=W), oview, OP.mult)
                  nc.gpsimd.dma_start(
                      out_d[obase:obase + rows].rearrange("r g w -> r (g w)"), O[:])

              # ---- software-pipelined chunk schedule ----
              # emission order: s4(k) | tail(k-1) | pairs01(k) | s4(k+1) |
              #                 pairs23(k) | tail(k) | ...
              prev = None
              pend = None       # (ch, st, htiles, csm) with pairs23 pending
              for ch in range(NCH):
                  st = phase_s4(ch)
                  if pend is not None:
                      phase_pairs(pend[1], range(2, 4), pend[2], pend[3],
                                  _btm=_btlist[pend[0]])
                      prev = (pend[0], pend[1], (pend[2], pend[3]))
                      pend = None
                  if prev is not None:
                      phase_tail(prev[0], prev[1], prev[2])
                      if prev[0] == 3:
                          mx_path_s1()
                          epi01()
                      if prev[0] == 4:
                          s_gate(S1, gb1, 128, 0, "a")
                      prev = None
                  htiles = []
                  csm = phase_pairs(st, range(0, 2), htiles, _btm=_btlist[ch],
                                    last=(ch == NCH - 1 and _SL01) or
                                         (ch == 0 and _SL0))
                  pend = (ch, st, htiles, csm)
              phase_pairs(pend[1], range(2, 4), pend[2], pend[3],
                          _btm=_btlist[pend[0]], last=True)
              phase_tail(pend[0], pend[1], (pend[2], pend[3]))
              mx_path_s2()

              # ---------- j2 epilogue (only piece left after last tail) ----
              avgr2 = epi.tile([G, 1], F32, tag="avgr2")
              nc.vector.tensor_reduce(avgr2[:], acc24[:, 16:24], AX.X, OP.add)
              nc.vector.tensor_tensor(ppool[:, 6:7], avgr2[:], invc[:, 2:3],
                                      OP.mult)
              z1p2 = pc.tile([4, 2], F32, tag="cp")
              nc.tensor.matmul(z1p2[:], lhsT=mlpw[:, 0:4], rhs=ppool[:, 6:8],
                               start=True, stop=True)
              nc.scalar.activation(z1ones[0:4, 2:3], z1p2[:, 0:1], AF.Relu,
                                   bias=mlpw[0:4, 12:13], scale=1.0)
              nc.scalar.activation(z1ones[0:4, 5:6], z1p2[:, 1:2], AF.Relu,
                                   bias=mlpw[0:4, 12:13], scale=1.0)
              gp2 = pc.tile([1, 8], F32, tag="cp")
              nc.tensor.matmul(gp2[:], lhsT=z1ones[0:5, 2:3], rhs=mlpw[0:5, 4:12],
                               start=True, stop=False)
              nc.tensor.matmul(gp2[:], lhsT=z1ones[0:5, 5:6], rhs=mlpw[0:5, 4:12],
                               start=False, stop=True)
              eg2 = epi.tile([1, 8], F32, tag="r1")
              nc.scalar.activation(eg2[:], gp2[:], AF.Exp, bias=0.0, scale=-1.0)
              egp2 = epi.tile([1, 8], F32, tag="avgr01")
              nc.vector.tensor_scalar_add(egp2[:], eg2[:], 1.0)
              chg2 = epi.tile([1, 8], BF16, tag="chg2")
              with nc.allow_low_precision(reason="bf16 channel gate"):
                  nc.vector.reciprocal(chg2[:], egp2[:])
              nc.gpsimd.partition_broadcast(gb2[:], chg2[:], channels=64)
              s_gate(S2, gb2, 64, 128, "b")
            for _it in range(_ITERS):
                _one_iter()

    nc.compile()
    return nc


def _host_inputs(inputs):
    """Build the 8 per-core input maps from the full problem inputs."""
    import ml_dtypes
    L = _f32(inputs["featuresL"])[0]          # [C,H,W]
    R = _f32(inputs["featuresR"])[0]
    W_in = _f32(inputs["W_in"])
    W_dt = _f32(inputs["W_dt"])
    b_dt = _f32(inputs["b_dt"])
    W_B = _f32(inputs["W_B"])
    W_C = _f32(inputs["W_C"])
    A = -np.exp(_f32(inputs["A_log"]))        # [E,S]
    D_skip = _f32(inputs["D_skip"])
    W_out = _f32(inputs["W_out"])
    W1, b1 = _f32(inputs["W1"]), _f32(inputs["b1"])
    W2, b2 = _f32(inputs["W2"]), _f32(inputs["b2"])
    w_sp, b_sp = _f32(inputs["w_sp"]), _f32(inputs["b_sp"])

    # stationary weights [64, 576]
    idx = np.arange(128) % 64
    wse = np.zeros((2 * C, 576), np.float32)
    wse[0:32, 0:128] = W_in[0::2][:, idx]
    wse[32:64, 128:256] = W_in[1::2][:, idx]
    wse[0:32, 256:384] = W_dt[0::2][:, idx]
    wse[32:64, 384:512] = W_dt[1::2][:, idx]
    W_comb = W_in @ (D_skip[:, None] * W_out)        # [64(c), G]
    wse[0:32, 512:520] = W_comb[0::2]
    wse[0:32, 520:528] = W_B[0::2]
    wse[0:32, 528:536] = W_C[0::2]
    wse[32:64, 544:552] = W_comb[1::2]
    wse[32:64, 552:560] = W_B[1::2]
    wse[32:64, 560:568] = W_C[1::2]

    # bf16 stationaries [128, 48]
    wbf = np.zeros((128, 48), np.float32)
    for row in range(128):
        cc, e = divmod(row, 64)
        for q in range(32):
            c2, g = q // 16, q % 16
            if g < 8 and cc == c2:
                wbf[row, q] = W_out[e, g]
    for p4 in range(4):
        for local in range(32):
            c2, g = local // 16, local % 16
            if g < 8:
                wbf[32 * p4 + local, 32 + g] = 1.0
    wbf[0:8, 40:48] = np.eye(8, dtype=np.float32)

    avec = np.zeros((128, 8), np.float32)
    avec[:, 0] = b_dt[idx]
    for p4 in range(4):
        cc = np.arange(128) // 64
        avec[:, 1 + p4] = A[idx, 2 * p4 + cc]

    wspv = np.zeros((128, 4), np.float32)
    wspv[:, 0] = w_sp[0] / G
    wspv[:, 1] = w_sp[1]
    wspv[:, 2] = np.float32(np.asarray(b_sp).reshape(-1)[0]) if np.asarray(b_sp).size else 0.0
    wspv[:, 3] = -wspv[:, 2]

    mlpv = np.zeros((G, 24), np.float32)
    mlpv[:, 0:4] = W1
    mlpv[0:4, 4:12] = W2
    mlpv[4, 4:12] = 2.0 * b2
    mlpv[0:4, 12] = b1
    mlpv[0, 16:19] = 1.0

    maps = []
    wi = np.arange(W)
    for k in range(NCORES):
        d0 = JD * k
        Rsh = np.zeros_like(R)
        if d0 > 0:
            Rsh[:, :, d0:] = R[:, :, :-d0]
        else:
            Rsh = R
        feat = np.zeros((C, 2 * FROW), np.float32)
        feat[:, PAD:PAD + HW] = L.reshape(C, HW)
        feat[:, FROW + PAD:] = Rsh.reshape(C, HW)

        umask = np.zeros((32, JD * W), np.float32)
        for j in range(JD):
            umask[:, j * W:(j + 1) * W] = (wi >= d0 + j).astype(np.float32)[None]

        # S-layout max-pool masks: S1 rows (j=0,1), S2 rows (j=2)
        mnegs = np.full((128, 2 * G * W), -1e30, np.float32)
        for j in range(2):
            row_mask = np.where(wi >= d0 + j, 0.0, -1e30)          # [W]
            mnegs[j * 64:(j + 1) * 64, 0:G * W] = np.tile(row_mask, G)[None]
        mnegs[0:64, G * W:2 * G * W] = np.tile(
            np.where(wi >= d0 + 2, 0.0, -1e30), G)[None]

        invc = np.zeros((G, JD), np.float32)
        for j in range(JD):
            invc[:, j] = 1.0 / (H * (W - (d0 + j)))

        maps.append({
            "feat": feat.astype(ml_dtypes.bfloat16),
            "wse": wse.astype(ml_dtypes.bfloat16),
            "wbf": wbf.astype(ml_dtypes.bfloat16),
            "avec": avec,
            "umask": umask.astype(ml_dtypes.bfloat16),
            "mnegs": mnegs.astype(ml_dtypes.bfloat16),
            "invc": invc,
            "wsp": wspv,
            "mlp": mlpv.astype(ml_dtypes.bfloat16),
        })
    return maps


def kernel(**inputs):
    from concourse.bass_utils import run_bass_kernel_spmd

    if "nc" not in _compiled:
        _compiled["nc"] = _build_program()
    nc = _compiled["nc"]

    maps = _host_inputs(inputs)
    res = run_bass_kernel_spmd(nc, maps, list(range(NCORES))).results

    vol = np.zeros((1, G, DV, H, W), np.float32)
    for k in range(NCORES):
        o = np.asarray(res[k]["out"], np.float32).reshape(JD, H, G, W)        # [j,h,g,w]
        vol[0, :, JD * k:JD * k + JD] = np.transpose(o, (2, 0, 1, 3))
    return vol

